# revision 5
# baseline (speedup 1.0000x reference)
"""Multi-Head Latent Attention kernel for 8 Trainium2 NeuronCores.

Sharding: data-parallel over (batch x strided query-block sets).
  core c: batch b = c // 4, idx = c % 4.
  Own query blocks (128 queries each): g = idx + 4*j, j in 0..3.
Each core redundantly computes latent/K/V for its batch (cross-core
collectives are ~30-60 GB/s here - far slower than recompute), so there is
zero cross-core communication. Causality is handled with a padded,
core-uniform block structure (NK(j) = 4j+4 key blocks for local block j)
plus per-core {0,1} multiplicative masks applied after exp - the SPMD
program is identical on all cores, only data differs.

All matmuls bf16 with fp32 PSUM accumulation; softmax runs without max
subtraction (scores are ~N(0,1) by construction, exp is safe in fp32).
"""

import math

import numpy as np
import ml_dtypes

import concourse.bacc as bacc
import concourse.mybir as mybir
import concourse.tile as tile

bf16 = ml_dtypes.bfloat16

EMB = 2048
HEADS = 16
D = 128          # head dim
L = 512          # latent dim
B, S = 2, 2048
NCORES = 8

EC = EMB // 128  # 16 e-chunks
LC = L // 128    # 4 l-chunks
QB = 4           # own q-blocks per core
NQ = QB * 128    # 512 own queries
SC = S // 512    # 4 s-chunks of 512
ST = S // 128    # 16 s-tiles of 128

NK = [4 * j + 4 for j in range(QB)]          # padded k-blocks per own block j
GRP_OFF = [0, 1, 3, 6]                        # mask group offset per j
NGRP = 10                                     # total [128,512] mask groups

_CACHE = {}


def build_program():
    nc = bacc.Bacc("TRN2", target_bir_lowering=False, debug=False)
    dt = mybir.dt

    xT = nc.dram_tensor("xT", [EMB, S], dt.bfloat16, kind="ExternalInput")
    xTq = nc.dram_tensor("xTq", [EMB, NQ], dt.bfloat16, kind="ExternalInput")
    wdT = nc.dram_tensor("wdT", [EMB, L], dt.bfloat16, kind="ExternalInput")
    wukT = nc.dram_tensor("wukT", [L, EMB], dt.bfloat16, kind="ExternalInput")
    wuvT = nc.dram_tensor("wuvT", [L, EMB], dt.bfloat16, kind="ExternalInput")
    wqT = nc.dram_tensor("wqT", [EMB, EMB], dt.bfloat16, kind="ExternalInput")
    woT = nc.dram_tensor("woT", [EMB, EMB], dt.bfloat16, kind="ExternalInput")
    bias = nc.dram_tensor("bias", [1, EMB], dt.bfloat16, kind="ExternalInput")
    masks = nc.dram_tensor("masks", [NGRP, 128, 512], dt.bfloat16, kind="ExternalInput")
    out = nc.dram_tensor("out", [NQ, EMB], dt.float32, kind="ExternalOutput")

    ident_t = nc.inline_tensor(np.eye(128, dtype=bf16), name="ident")
    ones_col_t = nc.inline_tensor(np.ones((128, 1), dtype=bf16), name="ones_col")
    ones_row_t = nc.inline_tensor(np.ones((1, 128), dtype=bf16), name="ones_row")

    scale = 1.0 / math.sqrt(D)

    import contextlib
    with tile.TileContext(nc) as tc, contextlib.ExitStack() as es:
        # ---- whole-kernel pools ----
        consts = es.enter_context(tc.tile_pool(name="consts", bufs=1, side="right"))
        p_qT = es.enter_context(tc.tile_pool(name="p_qT", bufs=1, side="right"))

        ident = consts.tile([128, 128], dt.bfloat16)
        nc.sync.dma_start(out=ident, in_=ident_t[:, :])
        ones_col = consts.tile([128, 1], dt.bfloat16)
        nc.sync.dma_start(out=ones_col, in_=ones_col_t[:, :])
        ones_row = consts.tile([1, 128], dt.bfloat16)
        nc.sync.dma_start(out=ones_row, in_=ones_row_t[:, :])
        bias_sb = consts.tile([1, EMB], dt.bfloat16)
        nc.sync.dma_start(out=bias_sb, in_=bias[:, :])

        # qT: f-tile h at cols h*NQ (within: own block j at j*128)
        qT_sb = p_qT.tile([128, HEADS * NQ], dt.bfloat16)

        if True:
            # ================= phase 0: projections =================
            lat_cm = tc.tile_pool(name="p_lat", bufs=1)
            p_lat = lat_cm.__enter__()
            with tc.tile_pool(name="ps0", bufs=6, space="PSUM") as ps0:

                latT_sb = p_lat.tile([128, LC * S], dt.bfloat16)  # l-chunk lc at cols lc*S

                # --- 0a: latentT[l, s] = wdT.T @ xT ---
                with tc.tile_pool(name="p_wd", bufs=1) as p_wd, \
                     tc.tile_pool(name="p_xt", bufs=2) as p_xt:
                    wd_sb = p_wd.tile([128, EC * L], dt.bfloat16)  # e-tile e at cols e*L
                    nc.sync.dma_start(
                        out=wd_sb.rearrange("p (c l) -> p c l", c=EC),
                        in_=wdT.rearrange("(c p) l -> p c l", p=128))
                    for sc in range(SC):
                        xt = p_xt.tile([128, EC * 512], dt.bfloat16, tag="xt")
                        nc.sync.dma_start(
                            out=xt.rearrange("p (c s) -> p c s", c=EC),
                            in_=xT[:, sc * 512:(sc + 1) * 512].rearrange(
                                "(c p) s -> p c s", p=128))
                        for lt in range(LC):
                            acc = ps0.tile([128, 512], dt.float32, tag="ps0")
                            for e in range(EC):
                                nc.tensor.matmul(
                                    acc,
                                    wd_sb[:, e * L + lt * 128: e * L + (lt + 1) * 128],
                                    xt[:, e * 512:(e + 1) * 512],
                                    start=(e == 0), stop=(e == EC - 1))
                            dst = latT_sb[:, lt * S + sc * 512: lt * S + (sc + 1) * 512]
                            if lt % 2 == 0:
                                nc.vector.tensor_copy(dst, acc)
                            else:
                                nc.scalar.copy(dst, acc)

                # --- 0d: qT[f, own q] = wqT.T @ xTq ---
                with tc.tile_pool(name="p_wq", bufs=1) as p_wq, \
                     tc.tile_pool(name="p_xtq", bufs=1) as p_xtq:
                    wq_sb = p_wq.tile([128, EC * EMB], dt.bfloat16)
                    nc.sync.dma_start(
                        out=wq_sb.rearrange("p (c f) -> p c f", c=EC),
                        in_=wqT.rearrange("(c p) f -> p c f", p=128))
                    xtq_sb = p_xtq.tile([128, EC * NQ], dt.bfloat16)
                    nc.sync.dma_start(
                        out=xtq_sb.rearrange("p (c q) -> p c q", c=EC),
                        in_=xTq.rearrange("(c p) q -> p c q", p=128))
                    for ft in range(EC):
                        acc = ps0.tile([128, NQ], dt.float32, tag="ps0")
                        for e in range(EC):
                            nc.tensor.matmul(
                                acc,
                                wq_sb[:, e * EMB + ft * 128: e * EMB + (ft + 1) * 128],
                                xtq_sb[:, e * NQ:(e + 1) * NQ],
                                start=(e == 0), stop=(e == EC - 1))
                        dst = qT_sb[:, ft * NQ:(ft + 1) * NQ]
                        if ft % 2 == 0:
                            nc.vector.tensor_copy(dst, acc)
                        else:
                            nc.scalar.copy(dst, acc)

                # --- 0b: kT[f, s] = wukT.T @ latentT ---
                p_kT = es.enter_context(tc.tile_pool(name="p_kT", bufs=1, side="right"))
                # kT: f-tile h at cols h*S, token s at +s
                kT_sb = p_kT.tile([128, HEADS * S], dt.bfloat16)
                with tc.tile_pool(name="p_wuk", bufs=1) as p_wuk:
                    wuk_sb = p_wuk.tile([128, LC * EMB], dt.bfloat16)
                    nc.sync.dma_start(
                        out=wuk_sb.rearrange("p (c f) -> p c f", c=LC),
                        in_=wukT.rearrange("(c p) f -> p c f", p=128))
                    for ft in range(EC):
                        for sc in range(SC):
                            acc = ps0.tile([128, 512], dt.float32, tag="ps0")
                            for lc in range(LC):
                                nc.tensor.matmul(
                                    acc,
                                    wuk_sb[:, lc * EMB + ft * 128: lc * EMB + (ft + 1) * 128],
                                    latT_sb[:, lc * S + sc * 512: lc * S + (sc + 1) * 512],
                                    start=(lc == 0), stop=(lc == LC - 1))
                            dst = kT_sb[:, ft * S + sc * 512: ft * S + (sc + 1) * 512]
                            if (ft + sc) % 2 == 0:
                                nc.vector.tensor_copy(dst, acc)
                            else:
                                nc.scalar.copy(dst, acc)

                # --- 0c: v[s, f] = latentT.T @ wuvT ---
                p_v = es.enter_context(tc.tile_pool(name="p_v", bufs=1, side="right"))
                # v: s-tile t at cols t*EMB, head h at +h*128 (partition = token)
                v_sb = p_v.tile([128, ST * EMB], dt.bfloat16)
                with tc.tile_pool(name="p_wuv", bufs=1) as p_wuv:
                    wuv_sb = p_wuv.tile([128, LC * EMB], dt.bfloat16)
                    nc.sync.dma_start(
                        out=wuv_sb.rearrange("p (c f) -> p c f", c=LC),
                        in_=wuvT.rearrange("(c p) f -> p c f", p=128))
                    for st in range(ST):
                        for fc in range(SC):
                            acc = ps0.tile([128, 512], dt.float32, tag="ps0")
                            for lc in range(LC):
                                nc.tensor.matmul(
                                    acc,
                                    latT_sb[:, lc * S + st * 128: lc * S + (st + 1) * 128],
                                    wuv_sb[:, lc * EMB + fc * 512: lc * EMB + (fc + 1) * 512],
                                    start=(lc == 0), stop=(lc == LC - 1))
                            dst = v_sb[:, st * EMB + fc * 512: st * EMB + (fc + 1) * 512]
                            if (st + fc) % 2 == 0:
                                nc.vector.tensor_copy(dst, acc)
                            else:
                                nc.scalar.copy(dst, acc)

            lat_cm.__exit__(None, None, None)

            # ================= phase 1: attention + out-proj =================
            with tc.tile_pool(name="p_masks", bufs=1) as p_masks, \
                 tc.tile_pool(name="p_wo", bufs=2) as p_wo, \
                 tc.tile_pool(name="p_attn", bufs=3) as p_attn, \
                 tc.tile_pool(name="p_ctx", bufs=4) as p_ctx, \
                 tc.tile_pool(name="p_small", bufs=4) as p_small, \
                 tc.tile_pool(name="p_out", bufs=2) as p_out, \
                 tc.tile_pool(name="ps_s", bufs=2, space="PSUM") as ps_s, \
                 tc.tile_pool(name="ps_cd", bufs=2, space="PSUM") as ps_cd, \
                 tc.tile_pool(name="ps_out", bufs=1, space="PSUM") as ps_out:

                masks_sb = p_masks.tile([128, NGRP * 512], dt.bfloat16)
                nc.sync.dma_start(
                    out=masks_sb.rearrange("p (g q) -> p g q", g=NGRP),
                    in_=masks.rearrange("g p q -> p g q"))

                for j in range(QB):
                    nk = NK[j]
                    out_ps = ps_out.tile([128, EMB], dt.float32, tag="out")
                    # seed bias: out_ps[m, f] = 1 * bias[f]
                    for fc in range(4):
                        nc.tensor.matmul(
                            out_ps[:, fc * 512:(fc + 1) * 512],
                            ones_row,
                            bias_sb[:, fc * 512:(fc + 1) * 512],
                            start=True, stop=False)
                    for h in range(HEADS):
                        wo_h = p_wo.tile([128, EMB], dt.bfloat16, tag="wo")
                        nc.sync.dma_start(
                            out=wo_h, in_=woT[h * 128:(h + 1) * 128, :])
                        cd = ps_cd.tile([128, 512], dt.float32, tag="cd")
                        for grp in range(nk // 4):
                            sT = ps_s.tile([128, 512], dt.float32, tag="s")
                            for s4 in range(4):
                                s = grp * 4 + s4
                                nc.tensor.matmul(
                                    sT[:, s4 * 128:(s4 + 1) * 128],
                                    kT_sb[:, h * S + s * 128: h * S + (s + 1) * 128],
                                    qT_sb[:, h * NQ + j * 128: h * NQ + (j + 1) * 128],
                                    start=(s4 == 0), stop=(s4 == 3),
                                    skip_group_check=True)
                            attn = p_attn.tile([128, 512], dt.bfloat16, tag="attn")
                            nc.scalar.activation(
                                attn, sT, mybir.ActivationFunctionType.Exp,
                                scale=scale)
                            gi = GRP_OFF[j] + grp
                            nc.vector.tensor_mul(
                                attn, attn, masks_sb[:, gi * 512:(gi + 1) * 512])
                            for s4 in range(4):
                                s = grp * 4 + s4
                                nc.tensor.matmul(
                                    cd[:, 0:128],
                                    attn[:, s4 * 128:(s4 + 1) * 128],
                                    v_sb[:, s * EMB + h * 128: s * EMB + (h + 1) * 128],
                                    start=(s == 0), stop=False,
                                    skip_group_check=True)
                                nc.tensor.matmul(
                                    cd[:, 128:129],
                                    attn[:, s4 * 128:(s4 + 1) * 128],
                                    ones_col,
                                    start=False, stop=(s == nk - 1),
                                    skip_group_check=True)
                        rcp = p_small.tile([128, 1], dt.float32, tag="rcp")
                        nc.vector.reciprocal(rcp, cd[:, 128:129])
                        ctxn = p_ctx.tile([128, 128], dt.bfloat16, tag="ctxn")
                        nc.vector.tensor_scalar_mul(ctxn, cd[:, 0:128], rcp)
                        ctxT_ps = ps_s.tile([128, 128], dt.bfloat16, tag="s")
                        nc.tensor.transpose(ctxT_ps, ctxn, ident)
                        ctxT = p_ctx.tile([128, 128], dt.bfloat16, tag="ctxT")
                        nc.scalar.copy(ctxT, ctxT_ps)
                        for fc in range(4):
                            nc.tensor.matmul(
                                out_ps[:, fc * 512:(fc + 1) * 512],
                                ctxT,
                                wo_h[:, fc * 512:(fc + 1) * 512],
                                start=False, stop=(h == HEADS - 1),
                                skip_group_check=True)
                    out_t = p_out.tile([128, EMB], dt.float32, tag="out_t")
                    nc.vector.tensor_copy(out_t[:, 0:1024], out_ps[:, 0:1024])
                    nc.scalar.copy(out_t[:, 1024:2048], out_ps[:, 1024:2048])
                    nc.sync.dma_start(
                        out=out[j * 128:(j + 1) * 128, :], in_=out_t)

    nc.finalize()
    return nc


def _shard_inputs(x, w_q, w_down, w_up_k, w_up_v, w_out, b_out):
    """Build the 8 per-core input maps (host-side layout prep)."""
    f32 = np.float32
    x = np.asarray(x, f32)
    wqT = np.ascontiguousarray(np.asarray(w_q, f32).T).astype(bf16)
    wdT = np.ascontiguousarray(np.asarray(w_down, f32).T).astype(bf16)
    wukT = np.ascontiguousarray(np.asarray(w_up_k, f32).T).astype(bf16)
    wuvT = np.ascontiguousarray(np.asarray(w_up_v, f32).T).astype(bf16)
    woT = np.ascontiguousarray(np.asarray(w_out, f32).T).astype(bf16)
    bias = np.asarray(b_out, f32).reshape(1, EMB).astype(bf16)

    xTs = [np.ascontiguousarray(x[b].T).astype(bf16) for b in range(B)]

    in_maps = []
    for c in range(NCORES):
        b, idx = c // 4, c % 4
        gs = [idx + 4 * j for j in range(QB)]
        xT = xTs[b]
        xTq = np.ascontiguousarray(
            np.concatenate([xT[:, g * 128:(g + 1) * 128] for g in gs], axis=1))
        # masks[10, 128, 512] per core: group gi covers slots s=grp*4+s4 of block j
        m = np.zeros((NGRP, 128, 512), dtype=bf16)
        tri = (np.arange(128)[:, None] <= np.arange(128)[None, :]).astype(bf16)
        onem = np.ones((128, 128), dtype=bf16)
        for j in range(QB):
            g = gs[j]
            for grp in range(NK[j] // 4):
                gi = GRP_OFF[j] + grp
                for s4 in range(4):
                    s = grp * 4 + s4
                    if s < g:
                        m[gi, :, s4 * 128:(s4 + 1) * 128] = onem
                    elif s == g:
                        m[gi, :, s4 * 128:(s4 + 1) * 128] = tri
        in_maps.append({
            "xT": xT, "xTq": xTq, "wdT": wdT, "wukT": wukT, "wuvT": wuvT,
            "wqT": wqT, "woT": woT, "bias": bias, "masks": m,
        })
    return in_maps


def _unshard(results, dtype):
    out = np.zeros((B, S, EMB), dtype=np.float32)
    for c in range(NCORES):
        b, idx = c // 4, c % 4
        o = results[c]["out"]
        for j in range(QB):
            g = idx + 4 * j
            out[b, g * 128:(g + 1) * 128, :] = o[j * 128:(j + 1) * 128, :]
    return out.astype(dtype)


def kernel(x, w_q, w_down, w_up_k, w_up_v, w_out, b_out):
    from concourse.bass_utils import run_bass_kernel_spmd
    if "nc" not in _CACHE:
        _CACHE["nc"] = build_program()
    nc = _CACHE["nc"]
    in_maps = _shard_inputs(x, w_q, w_down, w_up_k, w_up_v, w_out, b_out)
    res = run_bass_kernel_spmd(nc, in_maps, list(range(NCORES)))
    return _unshard(res.results, np.asarray(x).dtype)


if __name__ == "__main__":
    import reference
    inputs = {k: np.asarray(v) for k, v in reference.setup_inputs().items()}
    got = kernel(**inputs)
    want = np.asarray(reference.reference(**inputs))
    err = np.abs(got - want)
    print("absmax rel err:", err.max() / np.abs(want).max())


# revision 7
# speedup vs baseline: 1.2430x; 1.2430x over previous
"""Multi-Head Latent Attention kernel for 8 Trainium2 NeuronCores.

Sharding: data-parallel over (batch x strided query-block sets).
  core c: batch b = c // 4, idx = c % 4.
  Own query blocks (128 queries each): g = idx + 4*j, j in 0..3.
Each core redundantly computes latent/K/V for its batch (cross-core
collectives are ~30-60 GB/s here - far slower than recompute), so there is
zero cross-core communication. Causality is handled with a padded,
core-uniform block structure (NK(j) = 4j+4 key blocks for local block j)
plus per-core {0,1} multiplicative masks applied after exp - the SPMD
program is identical on all cores, only data differs.

All matmuls bf16 with fp32 PSUM accumulation; softmax runs without max
subtraction (scores are ~N(0,1) by construction, exp is safe in fp32).
"""

import math

import numpy as np
import ml_dtypes

import concourse.bacc as bacc
import concourse.mybir as mybir
import concourse.tile as tile

bf16 = ml_dtypes.bfloat16

EMB = 2048
HEADS = 16
D = 128          # head dim
L = 512          # latent dim
B, S = 2, 2048
NCORES = 8

EC = EMB // 128  # 16 e-chunks
LC = L // 128    # 4 l-chunks
QB = 4           # own q-blocks per core
NQ = QB * 128    # 512 own queries
SC = S // 512    # 4 s-chunks of 512
ST = S // 128    # 16 s-tiles of 128

NK = [4 * j + 4 for j in range(QB)]          # padded k-blocks per own block j
GRP_OFF = [0, 1, 3, 6]                        # mask group offset per j
NGRP = 10                                     # total [128,512] mask groups

_CACHE = {}


def build_program():
    nc = bacc.Bacc("TRN2", target_bir_lowering=False, debug=False)
    dt = mybir.dt

    xT = nc.dram_tensor("xT", [EMB, S], dt.bfloat16, kind="ExternalInput")
    xTq = nc.dram_tensor("xTq", [EMB, NQ], dt.bfloat16, kind="ExternalInput")
    wdT = nc.dram_tensor("wdT", [EMB, L], dt.bfloat16, kind="ExternalInput")
    wukT = nc.dram_tensor("wukT", [L, EMB], dt.bfloat16, kind="ExternalInput")
    wuvT = nc.dram_tensor("wuvT", [L, EMB], dt.bfloat16, kind="ExternalInput")
    wqT = nc.dram_tensor("wqT", [EMB, EMB], dt.bfloat16, kind="ExternalInput")
    woT = nc.dram_tensor("woT", [EMB, EMB], dt.bfloat16, kind="ExternalInput")
    bias = nc.dram_tensor("bias", [1, EMB], dt.bfloat16, kind="ExternalInput")
    masks = nc.dram_tensor("masks", [NGRP, 128, 512], dt.bfloat16, kind="ExternalInput")
    out = nc.dram_tensor("out", [NQ, EMB], dt.float32, kind="ExternalOutput")

    ident_t = nc.inline_tensor(np.eye(128, dtype=bf16), name="ident")
    ones_row_t = nc.inline_tensor(np.ones((1, 128), dtype=bf16), name="ones_row")

    scale = 1.0 / math.sqrt(D)
    import contextlib

    with tile.TileContext(nc) as tc, contextlib.ExitStack() as es:
        # ---- persistent (right-side) pools ----
        consts = es.enter_context(tc.tile_pool(name="consts", bufs=1, side="right"))
        p_qT = es.enter_context(tc.tile_pool(name="p_qT", bufs=1, side="right"))

        ident = consts.tile([128, 128], dt.bfloat16)
        nc.sync.dma_start(out=ident, in_=ident_t[:, :])
        ones_row = consts.tile([1, 128], dt.bfloat16)
        nc.sync.dma_start(out=ones_row, in_=ones_row_t[:, :])
        bias_sb = consts.tile([1, EMB], dt.bfloat16)
        nc.sync.dma_start(out=bias_sb, in_=bias[:, :])

        # qT: f-tile h at cols h*NQ (within: own block j at j*128)
        qT_sb = p_qT.tile([128, HEADS * NQ], dt.bfloat16)

        # ============ phase 0: projections ============
        lat_cm = tc.tile_pool(name="p_lat", bufs=1)
        p_lat = lat_cm.__enter__()
        latT_sb = p_lat.tile([128, LC * S], dt.bfloat16)  # l-chunk lc at cols lc*S

        wuv_cm = tc.tile_pool(name="p_wuv", bufs=1)
        p_wuv = wuv_cm.__enter__()
        wuv_sb = p_wuv.tile([128, LC * EMB], dt.bfloat16)

        wuk_cm = tc.tile_pool(name="p_wuk", bufs=1)
        p_wuk = wuk_cm.__enter__()
        wuk_sb = p_wuk.tile([128, LC * EMB], dt.bfloat16)

        wq_cm = tc.tile_pool(name="p_wq", bufs=1)
        p_wq = wq_cm.__enter__()
        wq_sb = p_wq.tile([128, EC * EMB], dt.bfloat16, tag="wq")
        xtq_sb = p_wq.tile([128, EC * NQ], dt.bfloat16, tag="xtq")

        ps0_cm = tc.tile_pool(name="ps0", bufs=4, space="PSUM")
        ps0 = ps0_cm.__enter__()
        if True:
            # --- 0a: latentT[l, s] = wdT.T @ xT ---
            with tc.tile_pool(name="p_wd", bufs=1) as p_wd, \
                 tc.tile_pool(name="p_xt", bufs=2) as p_xt:
                wd_sb = p_wd.tile([128, EC * L], dt.bfloat16)
                # issue 0a-critical DMAs first, then prefetch the later weights
                nc.sync.dma_start(
                    out=wd_sb.rearrange("p (c l) -> p c l", c=EC),
                    in_=wdT.rearrange("(c p) l -> p c l", p=128))
                xts = []
                for hc in range(2 * SC):
                    xt = p_xt.tile([128, EC * 256], dt.bfloat16, tag="xt")
                    nc.sync.dma_start(
                        out=xt.rearrange("p (c s) -> p c s", c=EC),
                        in_=xT[:, hc * 256:(hc + 1) * 256].rearrange(
                            "(c p) s -> p c s", p=128))
                    if hc == 0:
                        # prefetches (overlap with 0a/0d compute)
                        nc.sync.dma_start(
                            out=xtq_sb.rearrange("p (c q) -> p c q", c=EC),
                            in_=xTq.rearrange("(c p) q -> p c q", p=128))
                        nc.sync.dma_start(
                            out=wq_sb.rearrange("p (c f) -> p c f", c=EC),
                            in_=wqT.rearrange("(c p) f -> p c f", p=128))
                        nc.sync.dma_start(
                            out=wuk_sb.rearrange("p (c f) -> p c f", c=LC),
                            in_=wukT.rearrange("(c p) f -> p c f", p=128))
                        nc.sync.dma_start(
                            out=wuv_sb.rearrange("p (c f) -> p c f", c=LC),
                            in_=wuvT.rearrange("(c p) f -> p c f", p=128))
                    for lt in range(LC):
                        acc = ps0.tile([128, 256], dt.float32, tag="ps0")
                        for e in range(EC):
                            nc.tensor.matmul(
                                acc,
                                wd_sb[:, e * L + lt * 128: e * L + (lt + 1) * 128],
                                xt[:, e * 256:(e + 1) * 256],
                                start=(e == 0), stop=(e == EC - 1))
                        dst = latT_sb[:, lt * S + hc * 256: lt * S + (hc + 1) * 256]
                        if lt % 2 == 0:
                            nc.vector.tensor_copy(dst, acc)
                        else:
                            nc.scalar.copy(dst, acc)

            # --- 0d: qT[f, own q] = wqT.T @ xTq ---
            for ft in range(EC):
                acc = ps0.tile([128, NQ], dt.float32, tag="ps0")
                for e in range(EC):
                    nc.tensor.matmul(
                        acc,
                        wq_sb[:, e * EMB + ft * 128: e * EMB + (ft + 1) * 128],
                        xtq_sb[:, e * NQ:(e + 1) * NQ],
                        start=(e == 0), stop=(e == EC - 1))
                dst = qT_sb[:, ft * NQ:(ft + 1) * NQ]
                if ft % 2 == 0:
                    nc.vector.tensor_copy(dst, acc)
                else:
                    nc.scalar.copy(dst, acc)
            wq_cm.__exit__(None, None, None)
            ps0_cm.__exit__(None, None, None)
            psa_cm = tc.tile_pool(name="ps_acc", bufs=1, space="PSUM")
            psa = psa_cm.__enter__()

            # --- 0b: kT[f, s] = wukT.T @ latentT ---
            p_kT = es.enter_context(tc.tile_pool(name="p_kT", bufs=1, side="right"))
            kT_sb = p_kT.tile([128, HEADS * S], dt.bfloat16)
            for ft in range(EC):
                for lc in range(LC):
                    accs = []
                    # weight-stationary across the 4 s-chunks
                    for sc in range(SC):
                        if lc == 0:
                            acc = psa.tile([128, 512], dt.float32, tag=f"a{sc}")
                            accs.append(acc)
                        else:
                            acc = kacc[sc]
                        nc.tensor.matmul(
                            acc,
                            wuk_sb[:, lc * EMB + ft * 128: lc * EMB + (ft + 1) * 128],
                            latT_sb[:, lc * S + sc * 512: lc * S + (sc + 1) * 512],
                            start=(lc == 0), stop=(lc == LC - 1))
                    if lc == 0:
                        kacc = accs
                for sc in range(SC):
                    dst = kT_sb[:, ft * S + sc * 512: ft * S + (sc + 1) * 512]
                    if (ft + sc) % 2 == 0:
                        nc.vector.tensor_copy(dst, kacc[sc])
                    else:
                        nc.scalar.copy(dst, kacc[sc])
            wuk_cm.__exit__(None, None, None)

            # --- 0c: v[s, (h,129)] = latentT.T @ wuvT  (+ ones col per head) ---
            p_v = es.enter_context(tc.tile_pool(name="p_v", bufs=1, side="right"))
            v_sb = p_v.tile([128, ST * HEADS * (D + 1)], dt.bfloat16)
            nc.vector.memset(v_sb, 1.0)
            for st in range(ST):
                for lc in range(LC):
                    accs = []
                    for fc in range(SC):
                        if lc == 0:
                            acc = psa.tile([128, 512], dt.float32, tag=f"a{fc}")
                            accs.append(acc)
                        else:
                            acc = vacc[fc]
                        nc.tensor.matmul(
                            acc,
                            latT_sb[:, lc * S + st * 128: lc * S + (st + 1) * 128],
                            wuv_sb[:, lc * EMB + fc * 512: lc * EMB + (fc + 1) * 512],
                            start=(lc == 0), stop=(lc == LC - 1))
                    if lc == 0:
                        vacc = accs
                for fc in range(SC):
                    # scatter the 4 head-blocks of this 512-chunk into (h,129) layout
                    base = st * HEADS * (D + 1) + fc * 4 * (D + 1)
                    dst = v_sb[:, base: base + 4 * (D + 1)].rearrange(
                        "p (h w) -> p h w", h=4)[:, :, 0:D]
                    srcv = vacc[fc].rearrange("p (h w) -> p h w", h=4)
                    if (st + fc) % 2 == 0:
                        nc.vector.tensor_copy(dst, srcv)
                    else:
                        nc.scalar.copy(dst, srcv)
            wuv_cm.__exit__(None, None, None)
            lat_cm.__exit__(None, None, None)
            psa_cm.__exit__(None, None, None)

        # ============ phase 1: attention + out-proj ============
        with tc.tile_pool(name="p_masks", bufs=1) as p_masks, \
             tc.tile_pool(name="p_wo", bufs=4) as p_wo, \
             tc.tile_pool(name="p_attn", bufs=3) as p_attn, \
             tc.tile_pool(name="p_ctx", bufs=4) as p_ctx, \
             tc.tile_pool(name="p_small", bufs=4) as p_small, \
             tc.tile_pool(name="p_out", bufs=1) as p_out, \
             tc.tile_pool(name="ps_s", bufs=2, space="PSUM") as ps_s, \
             tc.tile_pool(name="ps_cd", bufs=2, space="PSUM") as ps_cd, \
             tc.tile_pool(name="ps_out", bufs=1, space="PSUM") as ps_out:

            masks_sb = p_masks.tile([128, NGRP * 512], dt.bfloat16)
            nc.sync.dma_start(
                out=masks_sb.rearrange("p (g q) -> p g q", g=NGRP),
                in_=masks.rearrange("g p q -> p g q"))

            for j in range(QB):
                nk = NK[j]
                out_ps = ps_out.tile([128, EMB], dt.float32, tag="out")
                for fc in range(4):
                    nc.tensor.matmul(
                        out_ps[:, fc * 512:(fc + 1) * 512],
                        ones_row,
                        bias_sb[:, fc * 512:(fc + 1) * 512],
                        start=True, stop=False, skip_group_check=True)
                ctxns = []

                def defer_outproj(h):
                    wo_h = wo_tiles[h]
                    ctxT_ps = ps_s.tile([128, 128], dt.bfloat16, tag="s")
                    nc.tensor.transpose(ctxT_ps, ctxns[h], ident)
                    ctxT = p_ctx.tile([128, 128], dt.bfloat16, tag="ctxT")
                    nc.scalar.copy(ctxT, ctxT_ps)
                    for fc in range(4):
                        nc.tensor.matmul(
                            out_ps[:, fc * 512:(fc + 1) * 512],
                            ctxT,
                            wo_h[:, fc * 512:(fc + 1) * 512],
                            start=False, stop=(h == HEADS - 1),
                            skip_group_check=True)

                wo_tiles = {}
                for h in range(HEADS):
                    wo_h = p_wo.tile([128, EMB], dt.bfloat16, tag="wo")
                    nc.sync.dma_start(out=wo_h, in_=woT[h * 128:(h + 1) * 128, :])
                    wo_tiles[h] = wo_h
                    cd = ps_cd.tile([128, 512], dt.float32, tag="cd")
                    for grp in range(nk // 4):
                        sT = ps_s.tile([128, 512], dt.float32, tag="s")
                        for s4 in range(4):
                            s = grp * 4 + s4
                            nc.tensor.matmul(
                                sT[:, s4 * 128:(s4 + 1) * 128],
                                kT_sb[:, h * S + s * 128: h * S + (s + 1) * 128],
                                qT_sb[:, h * NQ + j * 128: h * NQ + (j + 1) * 128],
                                start=(s4 == 0), stop=(s4 == 3),
                                skip_group_check=True)
                        attn = p_attn.tile([128, 512], dt.bfloat16, tag="attn")
                        nc.scalar.activation(
                            attn, sT, mybir.ActivationFunctionType.Exp,
                            scale=scale)
                        gi = GRP_OFF[j] + grp
                        nc.vector.tensor_mul(
                            attn, attn, masks_sb[:, gi * 512:(gi + 1) * 512])
                        for s4 in range(4):
                            s = grp * 4 + s4
                            # ctx (129 cols: 128 v cols + ones col -> denominator)
                            nc.tensor.matmul(
                                cd[:, 0:D + 1],
                                attn[:, s4 * 128:(s4 + 1) * 128],
                                v_sb[:, (s * HEADS + h) * (D + 1):
                                     (s * HEADS + h + 1) * (D + 1)],
                                start=(s == 0), stop=(s == nk - 1),
                                skip_group_check=True)
                    rcp = p_small.tile([128, 1], dt.float32, tag="rcp")
                    nc.vector.reciprocal(rcp, cd[:, D:D + 1])
                    ctxn = p_ctx.tile([128, 128], dt.bfloat16, tag="ctxn")
                    nc.vector.tensor_scalar_mul(ctxn, cd[:, 0:D], rcp)
                    ctxns.append(ctxn)
                    if h >= 1:
                        defer_outproj(h - 1)
                defer_outproj(HEADS - 1)
                out_t = p_out.tile([128, EMB], dt.float32, tag="out_t")
                nc.vector.tensor_copy(out_t[:, 0:1024], out_ps[:, 0:1024])
                nc.scalar.copy(out_t[:, 1024:2048], out_ps[:, 1024:2048])
                nc.sync.dma_start(
                    out=out[j * 128:(j + 1) * 128, :], in_=out_t)

    nc.finalize()
    return nc


def _shard_inputs(x, w_q, w_down, w_up_k, w_up_v, w_out, b_out):
    """Build the 8 per-core input maps (host-side layout prep)."""
    f32 = np.float32
    x = np.asarray(x, f32)
    wqT = np.ascontiguousarray(np.asarray(w_q, f32).T).astype(bf16)
    wdT = np.ascontiguousarray(np.asarray(w_down, f32).T).astype(bf16)
    wukT = np.ascontiguousarray(np.asarray(w_up_k, f32).T).astype(bf16)
    wuvT = np.ascontiguousarray(np.asarray(w_up_v, f32).T).astype(bf16)
    woT = np.ascontiguousarray(np.asarray(w_out, f32).T).astype(bf16)
    bias = np.asarray(b_out, f32).reshape(1, EMB).astype(bf16)

    xTs = [np.ascontiguousarray(x[b].T).astype(bf16) for b in range(B)]

    in_maps = []
    for c in range(NCORES):
        b, idx = c // 4, c % 4
        gs = [idx + 4 * j for j in range(QB)]
        xT = xTs[b]
        xTq = np.ascontiguousarray(
            np.concatenate([xT[:, g * 128:(g + 1) * 128] for g in gs], axis=1))
        # masks[10, 128, 512] per core: group gi covers slots s=grp*4+s4 of block j
        m = np.zeros((NGRP, 128, 512), dtype=bf16)
        tri = (np.arange(128)[:, None] <= np.arange(128)[None, :]).astype(bf16)
        onem = np.ones((128, 128), dtype=bf16)
        for j in range(QB):
            g = gs[j]
            for grp in range(NK[j] // 4):
                gi = GRP_OFF[j] + grp
                for s4 in range(4):
                    s = grp * 4 + s4
                    if s < g:
                        m[gi, :, s4 * 128:(s4 + 1) * 128] = onem
                    elif s == g:
                        m[gi, :, s4 * 128:(s4 + 1) * 128] = tri
        in_maps.append({
            "xT": xT, "xTq": xTq, "wdT": wdT, "wukT": wukT, "wuvT": wuvT,
            "wqT": wqT, "woT": woT, "bias": bias, "masks": m,
        })
    return in_maps


def _unshard(results, dtype):
    out = np.zeros((B, S, EMB), dtype=np.float32)
    for c in range(NCORES):
        b, idx = c // 4, c % 4
        o = results[c]["out"]
        for j in range(QB):
            g = idx + 4 * j
            out[b, g * 128:(g + 1) * 128, :] = o[j * 128:(j + 1) * 128, :]
    return out.astype(dtype)


def kernel(x, w_q, w_down, w_up_k, w_up_v, w_out, b_out):
    from concourse.bass_utils import run_bass_kernel_spmd
    if "nc" not in _CACHE:
        _CACHE["nc"] = build_program()
    nc = _CACHE["nc"]
    in_maps = _shard_inputs(x, w_q, w_down, w_up_k, w_up_v, w_out, b_out)
    res = run_bass_kernel_spmd(nc, in_maps, list(range(NCORES)))
    return _unshard(res.results, np.asarray(x).dtype)


if __name__ == "__main__":
    import reference
    inputs = {k: np.asarray(v) for k, v in reference.setup_inputs().items()}
    got = kernel(**inputs)
    want = np.asarray(reference.reference(**inputs))
    err = np.abs(got - want)
    print("absmax rel err:", err.max() / np.abs(want).max())


# revision 8
# speedup vs baseline: 1.2552x; 1.0099x over previous
"""Multi-Head Latent Attention kernel for 8 Trainium2 NeuronCores.

Sharding: data-parallel over (batch x strided query-block sets).
  core c: batch b = c // 4, idx = c % 4.
  Own query blocks (128 queries each): g = idx + 4*j, j in 0..3.
Each core redundantly computes latent/K/V for its batch (cross-core
collectives are ~30-60 GB/s here - far slower than recompute), so there is
zero cross-core communication. Causality is handled with a padded,
core-uniform block structure (NK(j) = 4j+4 key blocks for local block j)
plus per-core {0,1} multiplicative masks applied after exp - the SPMD
program is identical on all cores, only data differs.

All matmuls bf16 with fp32 PSUM accumulation; softmax runs without max
subtraction (scores are ~N(0,1) by construction, exp is safe in fp32).
"""

import math

import numpy as np
import ml_dtypes

import concourse.bacc as bacc
import concourse.mybir as mybir
import concourse.tile as tile

bf16 = ml_dtypes.bfloat16

EMB = 2048
HEADS = 16
D = 128          # head dim
L = 512          # latent dim
B, S = 2, 2048
NCORES = 8

EC = EMB // 128  # 16 e-chunks
LC = L // 128    # 4 l-chunks
QB = 4           # own q-blocks per core
NQ = QB * 128    # 512 own queries
SC = S // 512    # 4 s-chunks of 512
ST = S // 128    # 16 s-tiles of 128

NK = [4 * j + 4 for j in range(QB)]          # padded k-blocks per own block j
GRP_OFF = [0, 1, 3, 6]                        # mask group offset per j
NGRP = 10                                     # total [128,512] mask groups

_CACHE = {}


def build_program():
    nc = bacc.Bacc("TRN2", target_bir_lowering=False, debug=False)
    dt = mybir.dt

    xT = nc.dram_tensor("xT", [EMB, S], dt.bfloat16, kind="ExternalInput")
    xTq = nc.dram_tensor("xTq", [EMB, NQ], dt.bfloat16, kind="ExternalInput")
    wdT = nc.dram_tensor("wdT", [EMB, L], dt.bfloat16, kind="ExternalInput")
    wukT = nc.dram_tensor("wukT", [L, EMB], dt.bfloat16, kind="ExternalInput")
    wuvT = nc.dram_tensor("wuvT", [L, EMB], dt.bfloat16, kind="ExternalInput")
    wqT = nc.dram_tensor("wqT", [EMB, EMB], dt.bfloat16, kind="ExternalInput")
    woT = nc.dram_tensor("woT", [EMB, EMB], dt.bfloat16, kind="ExternalInput")
    bias = nc.dram_tensor("bias", [1, EMB], dt.bfloat16, kind="ExternalInput")
    masks = nc.dram_tensor("masks", [NGRP, 128, 512], dt.bfloat16, kind="ExternalInput")
    out = nc.dram_tensor("out", [NQ, EMB], dt.float32, kind="ExternalOutput")

    ident_t = nc.inline_tensor(np.eye(128, dtype=bf16), name="ident")
    ones_row_t = nc.inline_tensor(np.ones((1, 128), dtype=bf16), name="ones_row")

    scale = 1.0 / math.sqrt(D)
    import contextlib

    with tile.TileContext(nc) as tc, contextlib.ExitStack() as es:
        # ---- persistent (right-side) pools ----
        consts = es.enter_context(tc.tile_pool(name="consts", bufs=1, side="right"))
        p_qT = es.enter_context(tc.tile_pool(name="p_qT", bufs=1, side="right"))

        ident = consts.tile([128, 128], dt.bfloat16)
        nc.sync.dma_start(out=ident, in_=ident_t[:, :])
        ones_row = consts.tile([1, 128], dt.bfloat16)
        nc.sync.dma_start(out=ones_row, in_=ones_row_t[:, :])
        bias_sb = consts.tile([1, EMB], dt.bfloat16)
        nc.sync.dma_start(out=bias_sb, in_=bias[:, :])

        # qT: f-tile h at cols h*NQ (within: own block j at j*128)
        qT_sb = p_qT.tile([128, HEADS * NQ], dt.bfloat16)

        # ============ phase 0: projections ============
        lat_cm = tc.tile_pool(name="p_lat", bufs=1)
        p_lat = lat_cm.__enter__()
        latT_sb = p_lat.tile([128, LC * S], dt.bfloat16)  # l-chunk lc at cols lc*S

        wuv_cm = tc.tile_pool(name="p_wuv", bufs=1)
        p_wuv = wuv_cm.__enter__()
        wuv_sb = p_wuv.tile([128, LC * EMB], dt.bfloat16)

        wuk_cm = tc.tile_pool(name="p_wuk", bufs=1)
        p_wuk = wuk_cm.__enter__()
        wuk_sb = p_wuk.tile([128, LC * EMB], dt.bfloat16)

        wq_cm = tc.tile_pool(name="p_wq", bufs=1)
        p_wq = wq_cm.__enter__()
        wq_sb = p_wq.tile([128, EC * EMB], dt.bfloat16, tag="wq")
        xtq_sb = p_wq.tile([128, EC * NQ], dt.bfloat16, tag="xtq")

        ps0_cm = tc.tile_pool(name="ps0", bufs=4, space="PSUM")
        ps0 = ps0_cm.__enter__()
        if True:
            # --- 0a: latentT[l, s] = wdT.T @ xT ---
            with tc.tile_pool(name="p_wd", bufs=1) as p_wd, \
                 tc.tile_pool(name="p_xt", bufs=2) as p_xt:
                wd_sb = p_wd.tile([128, EC * L], dt.bfloat16)
                # issue 0a-critical DMAs first, then prefetch the later weights
                nc.sync.dma_start(
                    out=wd_sb.rearrange("p (c l) -> p c l", c=EC),
                    in_=wdT.rearrange("(c p) l -> p c l", p=128))
                xts = []
                for hc in range(2 * SC):
                    xt = p_xt.tile([128, EC * 256], dt.bfloat16, tag="xt")
                    nc.sync.dma_start(
                        out=xt.rearrange("p (c s) -> p c s", c=EC),
                        in_=xT[:, hc * 256:(hc + 1) * 256].rearrange(
                            "(c p) s -> p c s", p=128))
                    if hc == 2:
                        # prefetch 0d inputs (overlaps rest of 0a compute)
                        nc.sync.dma_start(
                            out=xtq_sb.rearrange("p (c q) -> p c q", c=EC),
                            in_=xTq.rearrange("(c p) q -> p c q", p=128))
                        nc.sync.dma_start(
                            out=wq_sb.rearrange("p (c f) -> p c f", c=EC),
                            in_=wqT.rearrange("(c p) f -> p c f", p=128))
                    for lt in range(LC):
                        acc = ps0.tile([128, 256], dt.float32, tag="ps0")
                        for e in range(EC):
                            nc.tensor.matmul(
                                acc,
                                wd_sb[:, e * L + lt * 128: e * L + (lt + 1) * 128],
                                xt[:, e * 256:(e + 1) * 256],
                                start=(e == 0), stop=(e == EC - 1))
                        dst = latT_sb[:, lt * S + hc * 256: lt * S + (hc + 1) * 256]
                        if lt % 2 == 0:
                            nc.vector.tensor_copy(dst, acc)
                        else:
                            nc.scalar.copy(dst, acc)

            # --- 0d: qT[f, own q] = wqT.T @ xTq ---
            for ft in range(EC):
                if ft == 4:
                    nc.sync.dma_start(
                        out=wuk_sb.rearrange("p (c f) -> p c f", c=LC),
                        in_=wukT.rearrange("(c p) f -> p c f", p=128))
                if ft == 8:
                    nc.sync.dma_start(
                        out=wuv_sb.rearrange("p (c f) -> p c f", c=LC),
                        in_=wuvT.rearrange("(c p) f -> p c f", p=128))
                acc = ps0.tile([128, NQ], dt.float32, tag="ps0")
                for e in range(EC):
                    nc.tensor.matmul(
                        acc,
                        wq_sb[:, e * EMB + ft * 128: e * EMB + (ft + 1) * 128],
                        xtq_sb[:, e * NQ:(e + 1) * NQ],
                        start=(e == 0), stop=(e == EC - 1))
                dst = qT_sb[:, ft * NQ:(ft + 1) * NQ]
                if ft % 2 == 0:
                    nc.vector.tensor_copy(dst, acc)
                else:
                    nc.scalar.copy(dst, acc)
            wq_cm.__exit__(None, None, None)
            ps0_cm.__exit__(None, None, None)
            psa_cm = tc.tile_pool(name="ps_acc", bufs=1, space="PSUM")
            psa = psa_cm.__enter__()

            # --- 0b: kT[f, s] = wukT.T @ latentT ---
            p_kT = es.enter_context(tc.tile_pool(name="p_kT", bufs=1, side="right"))
            kT_sb = p_kT.tile([128, HEADS * S], dt.bfloat16)
            for ft in range(EC):
                for lc in range(LC):
                    accs = []
                    # weight-stationary across the 4 s-chunks
                    for sc in range(SC):
                        if lc == 0:
                            acc = psa.tile([128, 512], dt.float32, tag=f"a{sc}")
                            accs.append(acc)
                        else:
                            acc = kacc[sc]
                        nc.tensor.matmul(
                            acc,
                            wuk_sb[:, lc * EMB + ft * 128: lc * EMB + (ft + 1) * 128],
                            latT_sb[:, lc * S + sc * 512: lc * S + (sc + 1) * 512],
                            start=(lc == 0), stop=(lc == LC - 1))
                    if lc == 0:
                        kacc = accs
                for sc in range(SC):
                    dst = kT_sb[:, ft * S + sc * 512: ft * S + (sc + 1) * 512]
                    if (ft + sc) % 2 == 0:
                        nc.vector.tensor_copy(dst, kacc[sc])
                    else:
                        nc.scalar.copy(dst, kacc[sc])
            wuk_cm.__exit__(None, None, None)

            # --- 0c: v[s, (h,129)] = latentT.T @ wuvT  (+ ones col per head) ---
            p_v = es.enter_context(tc.tile_pool(name="p_v", bufs=1, side="right"))
            v_sb = p_v.tile([128, ST * HEADS * (D + 1)], dt.bfloat16)
            nc.vector.memset(v_sb, 1.0)
            for st in range(ST):
                for lc in range(LC):
                    accs = []
                    for fc in range(SC):
                        if lc == 0:
                            acc = psa.tile([128, 512], dt.float32, tag=f"a{fc}")
                            accs.append(acc)
                        else:
                            acc = vacc[fc]
                        nc.tensor.matmul(
                            acc,
                            latT_sb[:, lc * S + st * 128: lc * S + (st + 1) * 128],
                            wuv_sb[:, lc * EMB + fc * 512: lc * EMB + (fc + 1) * 512],
                            start=(lc == 0), stop=(lc == LC - 1))
                    if lc == 0:
                        vacc = accs
                for fc in range(SC):
                    # scatter the 4 head-blocks of this 512-chunk into (h,129) layout
                    base = st * HEADS * (D + 1) + fc * 4 * (D + 1)
                    dst = v_sb[:, base: base + 4 * (D + 1)].rearrange(
                        "p (h w) -> p h w", h=4)[:, :, 0:D]
                    srcv = vacc[fc].rearrange("p (h w) -> p h w", h=4)
                    if (st + fc) % 2 == 0:
                        nc.vector.tensor_copy(dst, srcv)
                    else:
                        nc.scalar.copy(dst, srcv)
            wuv_cm.__exit__(None, None, None)
            lat_cm.__exit__(None, None, None)
            psa_cm.__exit__(None, None, None)

        # ============ phase 1: attention + out-proj ============
        with tc.tile_pool(name="p_masks", bufs=1) as p_masks, \
             tc.tile_pool(name="p_wo", bufs=4) as p_wo, \
             tc.tile_pool(name="p_attn", bufs=3) as p_attn, \
             tc.tile_pool(name="p_ctx", bufs=4) as p_ctx, \
             tc.tile_pool(name="p_small", bufs=4) as p_small, \
             tc.tile_pool(name="p_out", bufs=1) as p_out, \
             tc.tile_pool(name="ps_s", bufs=2, space="PSUM") as ps_s, \
             tc.tile_pool(name="ps_cd", bufs=2, space="PSUM") as ps_cd, \
             tc.tile_pool(name="ps_out", bufs=1, space="PSUM") as ps_out:

            masks_sb = p_masks.tile([128, NGRP * 512], dt.bfloat16)
            nc.sync.dma_start(
                out=masks_sb.rearrange("p (g q) -> p g q", g=NGRP),
                in_=masks.rearrange("g p q -> p g q"))

            for j in range(QB):
                nk = NK[j]
                out_ps = ps_out.tile([128, EMB], dt.float32, tag="out")
                for fc in range(4):
                    nc.tensor.matmul(
                        out_ps[:, fc * 512:(fc + 1) * 512],
                        ones_row,
                        bias_sb[:, fc * 512:(fc + 1) * 512],
                        start=True, stop=False, skip_group_check=True)
                ctxns = []

                def defer_outproj(h):
                    wo_h = wo_tiles[h]
                    ctxT_ps = ps_s.tile([128, 128], dt.bfloat16, tag="s")
                    nc.tensor.transpose(ctxT_ps, ctxns[h], ident)
                    ctxT = p_ctx.tile([128, 128], dt.bfloat16, tag="ctxT")
                    nc.scalar.copy(ctxT, ctxT_ps)
                    for fc in range(4):
                        nc.tensor.matmul(
                            out_ps[:, fc * 512:(fc + 1) * 512],
                            ctxT,
                            wo_h[:, fc * 512:(fc + 1) * 512],
                            start=False, stop=(h == HEADS - 1),
                            skip_group_check=True)

                wo_tiles = {}
                for h in range(HEADS):
                    wo_h = p_wo.tile([128, EMB], dt.bfloat16, tag="wo")
                    nc.sync.dma_start(out=wo_h, in_=woT[h * 128:(h + 1) * 128, :])
                    wo_tiles[h] = wo_h
                    cd = ps_cd.tile([128, 512], dt.float32, tag="cd")
                    for grp in range(nk // 4):
                        sT = ps_s.tile([128, 512], dt.float32, tag="s")
                        for s4 in range(4):
                            s = grp * 4 + s4
                            nc.tensor.matmul(
                                sT[:, s4 * 128:(s4 + 1) * 128],
                                kT_sb[:, h * S + s * 128: h * S + (s + 1) * 128],
                                qT_sb[:, h * NQ + j * 128: h * NQ + (j + 1) * 128],
                                start=(s4 == 0), stop=(s4 == 3),
                                skip_group_check=True)
                        attn = p_attn.tile([128, 512], dt.bfloat16, tag="attn")
                        nc.scalar.activation(
                            attn, sT, mybir.ActivationFunctionType.Exp,
                            scale=scale)
                        gi = GRP_OFF[j] + grp
                        nc.vector.tensor_mul(
                            attn, attn, masks_sb[:, gi * 512:(gi + 1) * 512])
                        for s4 in range(4):
                            s = grp * 4 + s4
                            # ctx (129 cols: 128 v cols + ones col -> denominator)
                            nc.tensor.matmul(
                                cd[:, 0:D + 1],
                                attn[:, s4 * 128:(s4 + 1) * 128],
                                v_sb[:, (s * HEADS + h) * (D + 1):
                                     (s * HEADS + h + 1) * (D + 1)],
                                start=(s == 0), stop=(s == nk - 1),
                                skip_group_check=True)
                    rcp = p_small.tile([128, 1], dt.float32, tag="rcp")
                    nc.vector.reciprocal(rcp, cd[:, D:D + 1])
                    ctxn = p_ctx.tile([128, 128], dt.bfloat16, tag="ctxn")
                    nc.vector.tensor_scalar_mul(ctxn, cd[:, 0:D], rcp)
                    ctxns.append(ctxn)
                    if h >= 1:
                        defer_outproj(h - 1)
                defer_outproj(HEADS - 1)
                out_t = p_out.tile([128, EMB], dt.float32, tag="out_t")
                nc.vector.tensor_copy(out_t[:, 0:1024], out_ps[:, 0:1024])
                nc.scalar.copy(out_t[:, 1024:2048], out_ps[:, 1024:2048])
                nc.sync.dma_start(
                    out=out[j * 128:(j + 1) * 128, :], in_=out_t)

    nc.finalize()
    return nc


def _shard_inputs(x, w_q, w_down, w_up_k, w_up_v, w_out, b_out):
    """Build the 8 per-core input maps (host-side layout prep)."""
    f32 = np.float32
    x = np.asarray(x, f32)
    wqT = np.ascontiguousarray(np.asarray(w_q, f32).T).astype(bf16)
    wdT = np.ascontiguousarray(np.asarray(w_down, f32).T).astype(bf16)
    wukT = np.ascontiguousarray(np.asarray(w_up_k, f32).T).astype(bf16)
    wuvT = np.ascontiguousarray(np.asarray(w_up_v, f32).T).astype(bf16)
    woT = np.ascontiguousarray(np.asarray(w_out, f32).T).astype(bf16)
    bias = np.asarray(b_out, f32).reshape(1, EMB).astype(bf16)

    xTs = [np.ascontiguousarray(x[b].T).astype(bf16) for b in range(B)]

    in_maps = []
    for c in range(NCORES):
        b, idx = c // 4, c % 4
        gs = [idx + 4 * j for j in range(QB)]
        xT = xTs[b]
        xTq = np.ascontiguousarray(
            np.concatenate([xT[:, g * 128:(g + 1) * 128] for g in gs], axis=1))
        # masks[10, 128, 512] per core: group gi covers slots s=grp*4+s4 of block j
        m = np.zeros((NGRP, 128, 512), dtype=bf16)
        tri = (np.arange(128)[:, None] <= np.arange(128)[None, :]).astype(bf16)
        onem = np.ones((128, 128), dtype=bf16)
        for j in range(QB):
            g = gs[j]
            for grp in range(NK[j] // 4):
                gi = GRP_OFF[j] + grp
                for s4 in range(4):
                    s = grp * 4 + s4
                    if s < g:
                        m[gi, :, s4 * 128:(s4 + 1) * 128] = onem
                    elif s == g:
                        m[gi, :, s4 * 128:(s4 + 1) * 128] = tri
        in_maps.append({
            "xT": xT, "xTq": xTq, "wdT": wdT, "wukT": wukT, "wuvT": wuvT,
            "wqT": wqT, "woT": woT, "bias": bias, "masks": m,
        })
    return in_maps


def _unshard(results, dtype):
    out = np.zeros((B, S, EMB), dtype=np.float32)
    for c in range(NCORES):
        b, idx = c // 4, c % 4
        o = results[c]["out"]
        for j in range(QB):
            g = idx + 4 * j
            out[b, g * 128:(g + 1) * 128, :] = o[j * 128:(j + 1) * 128, :]
    return out.astype(dtype)


def kernel(x, w_q, w_down, w_up_k, w_up_v, w_out, b_out):
    from concourse.bass_utils import run_bass_kernel_spmd
    if "nc" not in _CACHE:
        _CACHE["nc"] = build_program()
    nc = _CACHE["nc"]
    in_maps = _shard_inputs(x, w_q, w_down, w_up_k, w_up_v, w_out, b_out)
    res = run_bass_kernel_spmd(nc, in_maps, list(range(NCORES)))
    return _unshard(res.results, np.asarray(x).dtype)


if __name__ == "__main__":
    import reference
    inputs = {k: np.asarray(v) for k, v in reference.setup_inputs().items()}
    got = kernel(**inputs)
    want = np.asarray(reference.reference(**inputs))
    err = np.abs(got - want)
    print("absmax rel err:", err.max() / np.abs(want).max())


# revision 9
# speedup vs baseline: 1.3613x; 1.0845x over previous
"""Multi-Head Latent Attention kernel for 8 Trainium2 NeuronCores.

Sharding: data-parallel over (batch x strided query-block sets).
  core c: batch b = c // 4, idx = c % 4.
  Own query blocks (128 queries each): g = idx + 4*j, j in 0..3.
Each core redundantly computes latent/K/V for its batch (cross-core
collectives are ~30-60 GB/s here - far slower than recompute), so there is
zero cross-core communication. Causality is handled with a padded,
core-uniform block structure (NK(j) = 4j+4 key blocks for local block j)
plus per-core {0,1} multiplicative masks applied after exp - the SPMD
program is identical on all cores, only data differs.

All matmuls bf16 with fp32 PSUM accumulation; softmax runs without max
subtraction (scores are ~N(0,1) by construction, exp is safe in fp32).
"""

import math

import numpy as np
import ml_dtypes

import concourse.bacc as bacc
import concourse.mybir as mybir
import concourse.tile as tile

bf16 = ml_dtypes.bfloat16

EMB = 2048
HEADS = 16
D = 128          # head dim
L = 512          # latent dim
B, S = 2, 2048
NCORES = 8

EC = EMB // 128  # 16 e-chunks
LC = L // 128    # 4 l-chunks
QB = 4           # own q-blocks per core
NQ = QB * 128    # 512 own queries
SC = S // 512    # 4 s-chunks of 512
ST = S // 128    # 16 s-tiles of 128

NK = [4 * j + 4 for j in range(QB)]          # padded k-blocks per own block j
GRP_OFF = [0, 1, 3, 6]                        # mask group offset per j
NGRP = 10                                     # total [128,512] mask groups

_CACHE = {}


def build_program():
    nc = bacc.Bacc("TRN2", target_bir_lowering=False, debug=False)
    dt = mybir.dt

    xT = nc.dram_tensor("xT", [EMB, S], dt.bfloat16, kind="ExternalInput")
    xTq = nc.dram_tensor("xTq", [EMB, NQ], dt.bfloat16, kind="ExternalInput")
    wdT = nc.dram_tensor("wdT", [EMB, L], dt.bfloat16, kind="ExternalInput")
    wukT = nc.dram_tensor("wukT", [L, EMB], dt.bfloat16, kind="ExternalInput")
    wuvT = nc.dram_tensor("wuvT", [L, EMB], dt.bfloat16, kind="ExternalInput")
    wqT = nc.dram_tensor("wqT", [EMB, EMB], dt.bfloat16, kind="ExternalInput")
    woT = nc.dram_tensor("woT", [EMB, EMB], dt.bfloat16, kind="ExternalInput")
    bias = nc.dram_tensor("bias", [1, EMB], dt.bfloat16, kind="ExternalInput")
    masks = nc.dram_tensor("masks", [NGRP, 128, 512], dt.bfloat16, kind="ExternalInput")
    out = nc.dram_tensor("out", [NQ, EMB], dt.float32, kind="ExternalOutput")

    ident_t = nc.inline_tensor(np.eye(128, dtype=bf16), name="ident")
    ones_row_t = nc.inline_tensor(np.ones((1, 128), dtype=bf16), name="ones_row")

    scale = 1.0 / math.sqrt(D)
    import contextlib

    with tile.TileContext(nc) as tc, contextlib.ExitStack() as es:
        # ---- persistent (right-side) pools ----
        consts = es.enter_context(tc.tile_pool(name="consts", bufs=1, side="right"))
        p_qT = es.enter_context(tc.tile_pool(name="p_qT", bufs=1, side="right"))

        ident = consts.tile([128, 128], dt.bfloat16)
        nc.sync.dma_start(out=ident, in_=ident_t[:, :])
        ones_row = consts.tile([1, 128], dt.bfloat16)
        nc.sync.dma_start(out=ones_row, in_=ones_row_t[:, :])
        bias_sb = consts.tile([1, EMB], dt.bfloat16)
        nc.sync.dma_start(out=bias_sb, in_=bias[:, :])

        # qT: f-tile h at cols h*NQ (within: own block j at j*128)
        qT_sb = p_qT.tile([128, HEADS * NQ], dt.bfloat16)

        # ============ phase 0: projections ============
        lat_cm = tc.tile_pool(name="p_lat", bufs=1)
        p_lat = lat_cm.__enter__()
        latT_sb = p_lat.tile([128, LC * S], dt.bfloat16)  # l-chunk lc at cols lc*S

        wuv_cm = tc.tile_pool(name="p_wuv", bufs=1)
        p_wuv = wuv_cm.__enter__()
        wuv_sb = p_wuv.tile([128, LC * EMB], dt.bfloat16)

        wuk_cm = tc.tile_pool(name="p_wuk", bufs=1)
        p_wuk = wuk_cm.__enter__()
        wuk_sb = p_wuk.tile([128, LC * EMB], dt.bfloat16)

        wq_cm = tc.tile_pool(name="p_wq", bufs=1)
        p_wq = wq_cm.__enter__()
        wq_sb = p_wq.tile([128, EC * EMB], dt.bfloat16, tag="wq")
        xtq_sb = p_wq.tile([128, EC * NQ], dt.bfloat16, tag="xtq")

        ps0_cm = tc.tile_pool(name="ps0", bufs=4, space="PSUM")
        ps0 = ps0_cm.__enter__()
        if True:
            # --- 0a: latentT[l, s] = wdT.T @ xT ---
            with tc.tile_pool(name="p_wd", bufs=1) as p_wd, \
                 tc.tile_pool(name="p_xt", bufs=2) as p_xt:
                wd_sb = p_wd.tile([128, EC * L], dt.bfloat16)
                # issue 0a-critical DMAs first, then prefetch the later weights
                nc.sync.dma_start(
                    out=wd_sb.rearrange("p (c l) -> p c l", c=EC),
                    in_=wdT.rearrange("(c p) l -> p c l", p=128))
                xts = []
                for hc in range(2 * SC):
                    xt = p_xt.tile([128, EC * 256], dt.bfloat16, tag="xt")
                    nc.sync.dma_start(
                        out=xt.rearrange("p (c s) -> p c s", c=EC),
                        in_=xT[:, hc * 256:(hc + 1) * 256].rearrange(
                            "(c p) s -> p c s", p=128))
                    if hc == 2:
                        # prefetch 0d inputs (overlaps rest of 0a compute)
                        nc.gpsimd.dma_start(
                            out=xtq_sb.rearrange("p (c q) -> p c q", c=EC),
                            in_=xTq.rearrange("(c p) q -> p c q", p=128))
                        nc.gpsimd.dma_start(
                            out=wq_sb.rearrange("p (c f) -> p c f", c=EC),
                            in_=wqT.rearrange("(c p) f -> p c f", p=128))
                    for lt in range(LC):
                        acc = ps0.tile([128, 256], dt.float32, tag="ps0")
                        for e in range(EC):
                            nc.tensor.matmul(
                                acc,
                                wd_sb[:, e * L + lt * 128: e * L + (lt + 1) * 128],
                                xt[:, e * 256:(e + 1) * 256],
                                start=(e == 0), stop=(e == EC - 1))
                        dst = latT_sb[:, lt * S + hc * 256: lt * S + (hc + 1) * 256]
                        if lt % 2 == 0:
                            nc.vector.tensor_copy(dst, acc)
                        else:
                            nc.scalar.copy(dst, acc)

            # --- 0d: qT[f, own q] = wqT.T @ xTq ---
            for ft in range(EC):
                if ft == 4:
                    nc.gpsimd.dma_start(
                        out=wuk_sb.rearrange("p (c f) -> p c f", c=LC),
                        in_=wukT.rearrange("(c p) f -> p c f", p=128))
                if ft == 8:
                    nc.gpsimd.dma_start(
                        out=wuv_sb.rearrange("p (c f) -> p c f", c=LC),
                        in_=wuvT.rearrange("(c p) f -> p c f", p=128))
                acc = ps0.tile([128, NQ], dt.float32, tag="ps0")
                for e in range(EC):
                    nc.tensor.matmul(
                        acc,
                        wq_sb[:, e * EMB + ft * 128: e * EMB + (ft + 1) * 128],
                        xtq_sb[:, e * NQ:(e + 1) * NQ],
                        start=(e == 0), stop=(e == EC - 1))
                dst = qT_sb[:, ft * NQ:(ft + 1) * NQ]
                if ft % 2 == 0:
                    nc.vector.tensor_copy(dst, acc)
                else:
                    nc.scalar.copy(dst, acc)
            wq_cm.__exit__(None, None, None)
            ps0_cm.__exit__(None, None, None)
            psa_cm = tc.tile_pool(name="ps_acc", bufs=1, space="PSUM")
            psa = psa_cm.__enter__()

            # --- 0b: kT[f, s] = wukT.T @ latentT ---
            p_kT = es.enter_context(tc.tile_pool(name="p_kT", bufs=1, side="right"))
            kT_sb = p_kT.tile([128, HEADS * S], dt.bfloat16)
            for ft in range(EC):
                for lc in range(LC):
                    accs = []
                    # weight-stationary across the 4 s-chunks
                    for sc in range(SC):
                        if lc == 0:
                            acc = psa.tile([128, 512], dt.float32, tag=f"a{sc}")
                            accs.append(acc)
                        else:
                            acc = kacc[sc]
                        nc.tensor.matmul(
                            acc,
                            wuk_sb[:, lc * EMB + ft * 128: lc * EMB + (ft + 1) * 128],
                            latT_sb[:, lc * S + sc * 512: lc * S + (sc + 1) * 512],
                            start=(lc == 0), stop=(lc == LC - 1))
                    if lc == 0:
                        kacc = accs
                for sc in range(SC):
                    dst = kT_sb[:, ft * S + sc * 512: ft * S + (sc + 1) * 512]
                    if (ft + sc) % 2 == 0:
                        nc.vector.tensor_copy(dst, kacc[sc])
                    else:
                        nc.scalar.copy(dst, kacc[sc])
            wuk_cm.__exit__(None, None, None)

            # --- 0c: v[s, (h,129)] = latentT.T @ wuvT  (+ ones col per head) ---
            p_v = es.enter_context(tc.tile_pool(name="p_v", bufs=1, side="right"))
            p_masks = es.enter_context(
                tc.tile_pool(name="p_masks", bufs=1, side="right"))
            masks_sb = p_masks.tile([128, NGRP * 512], dt.bfloat16)
            nc.gpsimd.dma_start(
                out=masks_sb.rearrange("p (g q) -> p g q", g=NGRP),
                in_=masks.rearrange("g p q -> p g q"))
            v_sb = p_v.tile([128, ST * HEADS * (D + 1)], dt.bfloat16)
            nc.vector.memset(
                v_sb.rearrange("p (t w) -> p t w", w=D + 1)[:, :, D:D + 1], 1.0)
            for st in range(ST):
                for lc in range(LC):
                    accs = []
                    for fc in range(SC):
                        if lc == 0:
                            acc = psa.tile([128, 512], dt.float32, tag=f"a{fc}")
                            accs.append(acc)
                        else:
                            acc = vacc[fc]
                        nc.tensor.matmul(
                            acc,
                            latT_sb[:, lc * S + st * 128: lc * S + (st + 1) * 128],
                            wuv_sb[:, lc * EMB + fc * 512: lc * EMB + (fc + 1) * 512],
                            start=(lc == 0), stop=(lc == LC - 1))
                    if lc == 0:
                        vacc = accs
                for fc in range(SC):
                    # scatter the 4 head-blocks of this 512-chunk into (h,129) layout
                    base = st * HEADS * (D + 1) + fc * 4 * (D + 1)
                    dst = v_sb[:, base: base + 4 * (D + 1)].rearrange(
                        "p (h w) -> p h w", h=4)[:, :, 0:D]
                    srcv = vacc[fc].rearrange("p (h w) -> p h w", h=4)
                    if (st + fc) % 2 == 0:
                        nc.vector.tensor_copy(dst, srcv)
                    else:
                        nc.scalar.copy(dst, srcv)
            wuv_cm.__exit__(None, None, None)
            lat_cm.__exit__(None, None, None)
            psa_cm.__exit__(None, None, None)

        # ============ phase 1: attention + out-proj ============
        with tc.tile_pool(name="p_wo", bufs=4) as p_wo, \
             tc.tile_pool(name="p_attn", bufs=3) as p_attn, \
             tc.tile_pool(name="p_ctx", bufs=4) as p_ctx, \
             tc.tile_pool(name="p_small", bufs=4) as p_small, \
             tc.tile_pool(name="p_out", bufs=1) as p_out, \
             tc.tile_pool(name="ps_s", bufs=2, space="PSUM") as ps_s, \
             tc.tile_pool(name="ps_cd", bufs=2, space="PSUM") as ps_cd, \
             tc.tile_pool(name="ps_out", bufs=1, space="PSUM") as ps_out:

            for j in range(QB):
                nk = NK[j]
                out_ps = ps_out.tile([128, EMB], dt.float32, tag="out")
                for fc in range(4):
                    nc.tensor.matmul(
                        out_ps[:, fc * 512:(fc + 1) * 512],
                        ones_row,
                        bias_sb[:, fc * 512:(fc + 1) * 512],
                        start=True, stop=False, skip_group_check=True)
                ctxns = []

                def defer_outproj(h):
                    wo_h = wo_tiles[h]
                    ctxT_ps = ps_s.tile([128, 128], dt.bfloat16, tag="s")
                    nc.tensor.transpose(ctxT_ps, ctxns[h], ident)
                    ctxT = p_ctx.tile([128, 128], dt.bfloat16, tag="ctxT")
                    nc.scalar.copy(ctxT, ctxT_ps)
                    for fc in range(4):
                        nc.tensor.matmul(
                            out_ps[:, fc * 512:(fc + 1) * 512],
                            ctxT,
                            wo_h[:, fc * 512:(fc + 1) * 512],
                            start=False, stop=(h == HEADS - 1),
                            skip_group_check=True)

                wo_tiles = {}
                for h in range(HEADS):
                    wo_h = p_wo.tile([128, EMB], dt.bfloat16, tag="wo")
                    nc.sync.dma_start(out=wo_h, in_=woT[h * 128:(h + 1) * 128, :])
                    wo_tiles[h] = wo_h
                    cd = ps_cd.tile([128, 512], dt.float32, tag="cd")
                    for grp in range(nk // 4):
                        sT = ps_s.tile([128, 512], dt.float32, tag="s")
                        for s4 in range(4):
                            s = grp * 4 + s4
                            nc.tensor.matmul(
                                sT[:, s4 * 128:(s4 + 1) * 128],
                                kT_sb[:, h * S + s * 128: h * S + (s + 1) * 128],
                                qT_sb[:, h * NQ + j * 128: h * NQ + (j + 1) * 128],
                                start=(s4 == 0), stop=(s4 == 3),
                                skip_group_check=True)
                        attn = p_attn.tile([128, 512], dt.bfloat16, tag="attn")
                        nc.scalar.activation(
                            attn, sT, mybir.ActivationFunctionType.Exp,
                            scale=scale)
                        gi = GRP_OFF[j] + grp
                        nc.vector.tensor_mul(
                            attn, attn, masks_sb[:, gi * 512:(gi + 1) * 512])
                        for s4 in range(4):
                            s = grp * 4 + s4
                            # ctx (129 cols: 128 v cols + ones col -> denominator)
                            nc.tensor.matmul(
                                cd[:, 0:D + 1],
                                attn[:, s4 * 128:(s4 + 1) * 128],
                                v_sb[:, (s * HEADS + h) * (D + 1):
                                     (s * HEADS + h + 1) * (D + 1)],
                                start=(s == 0), stop=(s == nk - 1),
                                skip_group_check=True)
                    rcp = p_small.tile([128, 1], dt.float32, tag="rcp")
                    nc.vector.reciprocal(rcp, cd[:, D:D + 1])
                    ctxn = p_ctx.tile([128, 128], dt.bfloat16, tag="ctxn")
                    nc.vector.tensor_scalar_mul(ctxn, cd[:, 0:D], rcp)
                    ctxns.append(ctxn)
                    if h >= 1:
                        defer_outproj(h - 1)
                defer_outproj(HEADS - 1)
                out_t = p_out.tile([128, EMB], dt.float32, tag="out_t")
                nc.vector.tensor_copy(out_t[:, 0:1024], out_ps[:, 0:1024])
                nc.scalar.copy(out_t[:, 1024:2048], out_ps[:, 1024:2048])
                nc.sync.dma_start(
                    out=out[j * 128:(j + 1) * 128, :], in_=out_t)

    nc.finalize()
    return nc


def _shard_inputs(x, w_q, w_down, w_up_k, w_up_v, w_out, b_out):
    """Build the 8 per-core input maps (host-side layout prep)."""
    f32 = np.float32
    x = np.asarray(x, f32)
    wqT = np.ascontiguousarray(np.asarray(w_q, f32).T).astype(bf16)
    wdT = np.ascontiguousarray(np.asarray(w_down, f32).T).astype(bf16)
    wukT = np.ascontiguousarray(np.asarray(w_up_k, f32).T).astype(bf16)
    wuvT = np.ascontiguousarray(np.asarray(w_up_v, f32).T).astype(bf16)
    woT = np.ascontiguousarray(np.asarray(w_out, f32).T).astype(bf16)
    bias = np.asarray(b_out, f32).reshape(1, EMB).astype(bf16)

    xTs = [np.ascontiguousarray(x[b].T).astype(bf16) for b in range(B)]

    in_maps = []
    for c in range(NCORES):
        b, idx = c // 4, c % 4
        gs = [idx + 4 * j for j in range(QB)]
        xT = xTs[b]
        xTq = np.ascontiguousarray(
            np.concatenate([xT[:, g * 128:(g + 1) * 128] for g in gs], axis=1))
        # masks[10, 128, 512] per core: group gi covers slots s=grp*4+s4 of block j
        m = np.zeros((NGRP, 128, 512), dtype=bf16)
        tri = (np.arange(128)[:, None] <= np.arange(128)[None, :]).astype(bf16)
        onem = np.ones((128, 128), dtype=bf16)
        for j in range(QB):
            g = gs[j]
            for grp in range(NK[j] // 4):
                gi = GRP_OFF[j] + grp
                for s4 in range(4):
                    s = grp * 4 + s4
                    if s < g:
                        m[gi, :, s4 * 128:(s4 + 1) * 128] = onem
                    elif s == g:
                        m[gi, :, s4 * 128:(s4 + 1) * 128] = tri
        in_maps.append({
            "xT": xT, "xTq": xTq, "wdT": wdT, "wukT": wukT, "wuvT": wuvT,
            "wqT": wqT, "woT": woT, "bias": bias, "masks": m,
        })
    return in_maps


def _unshard(results, dtype):
    out = np.zeros((B, S, EMB), dtype=np.float32)
    for c in range(NCORES):
        b, idx = c // 4, c % 4
        o = results[c]["out"]
        for j in range(QB):
            g = idx + 4 * j
            out[b, g * 128:(g + 1) * 128, :] = o[j * 128:(j + 1) * 128, :]
    return out.astype(dtype)


def kernel(x, w_q, w_down, w_up_k, w_up_v, w_out, b_out):
    from concourse.bass_utils import run_bass_kernel_spmd
    if "nc" not in _CACHE:
        _CACHE["nc"] = build_program()
    nc = _CACHE["nc"]
    in_maps = _shard_inputs(x, w_q, w_down, w_up_k, w_up_v, w_out, b_out)
    res = run_bass_kernel_spmd(nc, in_maps, list(range(NCORES)))
    return _unshard(res.results, np.asarray(x).dtype)


if __name__ == "__main__":
    import reference
    inputs = {k: np.asarray(v) for k, v in reference.setup_inputs().items()}
    got = kernel(**inputs)
    want = np.asarray(reference.reference(**inputs))
    err = np.abs(got - want)
    print("absmax rel err:", err.max() / np.abs(want).max())


# revision 10
# speedup vs baseline: 1.3992x; 1.0279x over previous
"""Multi-Head Latent Attention kernel for 8 Trainium2 NeuronCores.

Sharding: data-parallel over (batch x strided query-block sets).
  core c: batch b = c // 4, idx = c % 4.
  Own query blocks (128 queries each): g = idx + 4*j, j in 0..3.
Each core redundantly computes latent/K/V for its batch (cross-core
collectives are ~30-60 GB/s here - far slower than recompute), so there is
zero cross-core communication. Causality is handled with a padded,
core-uniform block structure (NK(j) = 4j+4 key blocks for local block j)
plus per-core {0,1} multiplicative masks applied after exp - the SPMD
program is identical on all cores, only data differs.

All matmuls bf16 with fp32 PSUM accumulation; softmax runs without max
subtraction (scores are ~N(0,1) by construction, exp is safe in fp32).
"""

import math

import numpy as np
import ml_dtypes

import concourse.bacc as bacc
import concourse.mybir as mybir
import concourse.tile as tile

bf16 = ml_dtypes.bfloat16

EMB = 2048
HEADS = 16
D = 128          # head dim
L = 512          # latent dim
B, S = 2, 2048
NCORES = 8

EC = EMB // 128  # 16 e-chunks
LC = L // 128    # 4 l-chunks
QB = 4           # own q-blocks per core
NQ = QB * 128    # 512 own queries
SC = S // 512    # 4 s-chunks of 512
ST = S // 128    # 16 s-tiles of 128

NK = [4 * j + 4 for j in range(QB)]          # padded k-blocks per own block j
GRP_OFF = [0, 1, 3, 6]                        # mask group offset per j
NGRP = 10                                     # total [128,512] mask groups

_CACHE = {}


def build_program():
    nc = bacc.Bacc("TRN2", target_bir_lowering=False, debug=False)
    dt = mybir.dt

    xT = nc.dram_tensor("xT", [EMB, S], dt.bfloat16, kind="ExternalInput")
    xTq = nc.dram_tensor("xTq", [EMB, NQ], dt.bfloat16, kind="ExternalInput")
    wdT = nc.dram_tensor("wdT", [EMB, L], dt.bfloat16, kind="ExternalInput")
    wukT = nc.dram_tensor("wukT", [L, EMB], dt.bfloat16, kind="ExternalInput")
    wuvT = nc.dram_tensor("wuvT", [L, EMB], dt.bfloat16, kind="ExternalInput")
    # wq4[ftp, c, p, f]: f-tile pair ftp (256 f cols), e-chunk c, partition p
    wq4 = nc.dram_tensor("wq4", [EC // 2, EC, 128, 256], dt.bfloat16, kind="ExternalInput")
    woT = nc.dram_tensor("woT", [EMB, EMB], dt.bfloat16, kind="ExternalInput")
    bias = nc.dram_tensor("bias", [1, EMB], dt.bfloat16, kind="ExternalInput")
    masks = nc.dram_tensor("masks", [NGRP, 128, 512], dt.bfloat16, kind="ExternalInput")
    out = nc.dram_tensor("out", [NQ, EMB], dt.float32, kind="ExternalOutput")

    ident_t = nc.inline_tensor(np.eye(128, dtype=bf16), name="ident")
    ones_row_t = nc.inline_tensor(np.ones((1, 128), dtype=bf16), name="ones_row")

    scale = 1.0 / math.sqrt(D)
    import contextlib

    with tile.TileContext(nc) as tc, contextlib.ExitStack() as es:
        # ---- persistent (right-side) pools ----
        consts = es.enter_context(tc.tile_pool(name="consts", bufs=1, side="right"))
        p_qT = es.enter_context(tc.tile_pool(name="p_qT", bufs=1, side="right"))

        ident = consts.tile([128, 128], dt.bfloat16)
        nc.sync.dma_start(out=ident, in_=ident_t[:, :])
        ones_row = consts.tile([1, 128], dt.bfloat16)
        nc.sync.dma_start(out=ones_row, in_=ones_row_t[:, :])

        qT_sb = p_qT.tile([128, HEADS * NQ], dt.bfloat16)

        # left-stack pools, LIFO: latT > wuv > wuk > {xtq,wqs} > {wd,xt}
        lat_cm = tc.tile_pool(name="p_lat", bufs=1)
        p_lat = lat_cm.__enter__()
        latT_sb = p_lat.tile([128, LC * S], dt.bfloat16)

        wuv_cm = tc.tile_pool(name="p_wuv", bufs=1)
        p_wuv = wuv_cm.__enter__()
        wuv_sb = p_wuv.tile([128, LC * EMB], dt.bfloat16)

        wuk_cm = tc.tile_pool(name="p_wuk", bufs=1)
        p_wuk = wuk_cm.__enter__()
        wuk_sb = p_wuk.tile([128, LC * EMB], dt.bfloat16)

        wq_cm = tc.tile_pool(name="p_wq", bufs=2)
        p_wq = wq_cm.__enter__()
        xtq_sb = p_wq.tile([128, EC * NQ], dt.bfloat16, tag="xtq")

        ps0_cm = tc.tile_pool(name="ps0", bufs=4, space="PSUM")
        ps0 = ps0_cm.__enter__()

        # --- 0a: latentT[l, s] = wdT.T @ xT ---
        with tc.tile_pool(name="p_wd", bufs=1) as p_wd, \
             tc.tile_pool(name="p_xt", bufs=2) as p_xt:
            wd_sb = p_wd.tile([128, EC * L], dt.bfloat16)
            nc.sync.dma_start(
                out=wd_sb.rearrange("p (c l) -> p c l", c=EC),
                in_=wdT.rearrange("(c p) l -> p c l", p=128))
            for hc in range(2 * SC):
                xt = p_xt.tile([128, EC * 256], dt.bfloat16, tag="xt")
                nc.sync.dma_start(
                    out=xt.rearrange("p (c s) -> p c s", c=EC),
                    in_=xT[:, hc * 256:(hc + 1) * 256].rearrange(
                        "(c p) s -> p c s", p=128))
                if hc == 2:
                    nc.gpsimd.dma_start(
                        out=xtq_sb.rearrange("p (c q) -> p c q", c=EC),
                        in_=xTq.rearrange("(c p) q -> p c q", p=128))
                if hc == 5:
                    nc.gpsimd.dma_start(
                        out=wuk_sb.rearrange("p (c f) -> p c f", c=LC),
                        in_=wukT.rearrange("(c p) f -> p c f", p=128))
                for lt in range(LC):
                    acc = ps0.tile([128, 256], dt.float32, tag="ps0")
                    for e in range(EC):
                        nc.tensor.matmul(
                            acc,
                            wd_sb[:, e * L + lt * 128: e * L + (lt + 1) * 128],
                            xt[:, e * 256:(e + 1) * 256],
                            start=(e == 0), stop=(e == EC - 1))
                    dst = latT_sb[:, lt * S + hc * 256: lt * S + (hc + 1) * 256]
                    if lt % 2 == 0:
                        nc.vector.tensor_copy(dst, acc)
                    else:
                        nc.scalar.copy(dst, acc)

        # --- 0d: qT[f, own q] = wq.T @ xTq  (wq streamed in f-tile pairs) ---
        for ftp in range(EC // 2):
            wqs = p_wq.tile([128, EC * 256], dt.bfloat16, tag="wqs")
            nc.sync.dma_start(
                out=wqs.rearrange("p (c f) -> p c f", c=EC),
                in_=wq4[ftp].rearrange("c p f -> p c f"))
            if ftp == 4:
                nc.gpsimd.dma_start(
                    out=wuv_sb.rearrange("p (c f) -> p c f", c=LC),
                    in_=wuvT.rearrange("(c p) f -> p c f", p=128))
            for fi in range(2):
                ft = 2 * ftp + fi
                acc = ps0.tile([128, NQ], dt.float32, tag="ps0")
                for e in range(EC):
                    nc.tensor.matmul(
                        acc,
                        wqs[:, e * 256 + fi * 128: e * 256 + (fi + 1) * 128],
                        xtq_sb[:, e * NQ:(e + 1) * NQ],
                        start=(e == 0), stop=(e == EC - 1))
                dst = qT_sb[:, ft * NQ:(ft + 1) * NQ]
                if ft % 2 == 0:
                    nc.vector.tensor_copy(dst, acc)
                else:
                    nc.scalar.copy(dst, acc)
        wq_cm.__exit__(None, None, None)
        ps0_cm.__exit__(None, None, None)
        psa_cm = tc.tile_pool(name="ps_acc", bufs=1, space="PSUM")
        psa = psa_cm.__enter__()

        # --- 0b: kT[f, s] = wukT.T @ latentT (weight-stationary over s-chunks) ---
        p_kT = es.enter_context(tc.tile_pool(name="p_kT", bufs=1, side="right"))
        kT_sb = p_kT.tile([128, HEADS * S], dt.bfloat16)
        for ft in range(EC):
            for lc in range(LC):
                accs = []
                for sc in range(SC):
                    if lc == 0:
                        acc = psa.tile([128, 512], dt.float32, tag=f"a{sc}")
                        accs.append(acc)
                    else:
                        acc = kacc[sc]
                    nc.tensor.matmul(
                        acc,
                        wuk_sb[:, lc * EMB + ft * 128: lc * EMB + (ft + 1) * 128],
                        latT_sb[:, lc * S + sc * 512: lc * S + (sc + 1) * 512],
                        start=(lc == 0), stop=(lc == LC - 1))
                if lc == 0:
                    kacc = accs
            for sc in range(SC):
                dst = kT_sb[:, ft * S + sc * 512: ft * S + (sc + 1) * 512]
                if (ft + sc) % 2 == 0:
                    nc.vector.tensor_copy(dst, kacc[sc])
                else:
                    nc.scalar.copy(dst, kacc[sc])
        wuk_cm.__exit__(None, None, None)

        # --- 0c: v[s, (h,129)] = latentT.T @ wuvT (+ ones col per head) ---
        p_v = es.enter_context(tc.tile_pool(name="p_v", bufs=1, side="right"))
        p_masks = es.enter_context(tc.tile_pool(name="p_masks", bufs=1, side="right"))
        masks_sb = p_masks.tile([128, NGRP * 512], dt.bfloat16)
        nc.gpsimd.dma_start(
            out=masks_sb.rearrange("p (g q) -> p g q", g=NGRP),
            in_=masks.rearrange("g p q -> p g q"))
        v_sb = p_v.tile([128, ST * HEADS * (D + 1)], dt.bfloat16)
        nc.vector.memset(
            v_sb.rearrange("p (t w) -> p t w", w=D + 1)[:, :, D:D + 1], 1.0)
        for st in range(ST):
            for lc in range(LC):
                accs = []
                for fc in range(SC):
                    if lc == 0:
                        acc = psa.tile([128, 512], dt.float32, tag=f"a{fc}")
                        accs.append(acc)
                    else:
                        acc = vacc[fc]
                    nc.tensor.matmul(
                        acc,
                        latT_sb[:, lc * S + st * 128: lc * S + (st + 1) * 128],
                        wuv_sb[:, lc * EMB + fc * 512: lc * EMB + (fc + 1) * 512],
                        start=(lc == 0), stop=(lc == LC - 1))
                if lc == 0:
                    vacc = accs
            for fc in range(SC):
                base = st * HEADS * (D + 1) + fc * 4 * (D + 1)
                dst = v_sb[:, base: base + 4 * (D + 1)].rearrange(
                    "p (h w) -> p h w", h=4)[:, :, 0:D]
                srcv = vacc[fc].rearrange("p (h w) -> p h w", h=4)
                if (st + fc) % 2 == 0:
                    nc.vector.tensor_copy(dst, srcv)
                else:
                    nc.scalar.copy(dst, srcv)
        wuv_cm.__exit__(None, None, None)
        lat_cm.__exit__(None, None, None)
        psa_cm.__exit__(None, None, None)

        # ============ phase 1: attention + out-proj ============
        with tc.tile_pool(name="p_bias", bufs=1) as p_bias, \
             tc.tile_pool(name="p_wo", bufs=4) as p_wo, \
             tc.tile_pool(name="p_attn", bufs=3) as p_attn, \
             tc.tile_pool(name="p_ctx", bufs=4) as p_ctx, \
             tc.tile_pool(name="p_small", bufs=4) as p_small, \
             tc.tile_pool(name="p_out", bufs=1) as p_out, \
             tc.tile_pool(name="ps_s", bufs=2, space="PSUM") as ps_s, \
             tc.tile_pool(name="ps_cd", bufs=2, space="PSUM") as ps_cd, \
             tc.tile_pool(name="ps_out", bufs=1, space="PSUM") as ps_out:

            bias_sb = p_bias.tile([1, EMB], dt.bfloat16)
            nc.sync.dma_start(out=bias_sb, in_=bias[:, :])

            for j in range(QB):
                nk = NK[j]
                out_ps = ps_out.tile([128, EMB], dt.float32, tag="out")
                for fc in range(4):
                    nc.tensor.matmul(
                        out_ps[:, fc * 512:(fc + 1) * 512],
                        ones_row,
                        bias_sb[:, fc * 512:(fc + 1) * 512],
                        start=True, stop=False, skip_group_check=True)
                ctxns = []

                def defer_outproj(h):
                    wo_h = wo_tiles[h]
                    ctxT_ps = ps_s.tile([128, 128], dt.bfloat16, tag="s")
                    nc.tensor.transpose(ctxT_ps, ctxns[h], ident)
                    ctxT = p_ctx.tile([128, 128], dt.bfloat16, tag="ctxT")
                    nc.scalar.copy(ctxT, ctxT_ps)
                    for fc in range(4):
                        nc.tensor.matmul(
                            out_ps[:, fc * 512:(fc + 1) * 512],
                            ctxT,
                            wo_h[:, fc * 512:(fc + 1) * 512],
                            start=False, stop=(h == HEADS - 1),
                            skip_group_check=True)

                wo_tiles = {}
                for h in range(HEADS):
                    wo_h = p_wo.tile([128, EMB], dt.bfloat16, tag="wo")
                    nc.sync.dma_start(out=wo_h, in_=woT[h * 128:(h + 1) * 128, :])
                    wo_tiles[h] = wo_h
                    cd = ps_cd.tile([128, 512], dt.float32, tag="cd")
                    for grp in range(nk // 4):
                        sT = ps_s.tile([128, 512], dt.float32, tag="s")
                        for s4 in range(4):
                            s = grp * 4 + s4
                            nc.tensor.matmul(
                                sT[:, s4 * 128:(s4 + 1) * 128],
                                kT_sb[:, h * S + s * 128: h * S + (s + 1) * 128],
                                qT_sb[:, h * NQ + j * 128: h * NQ + (j + 1) * 128],
                                start=(s4 == 0), stop=(s4 == 3),
                                skip_group_check=True)
                        attn = p_attn.tile([128, 512], dt.bfloat16, tag="attn")
                        nc.scalar.activation(
                            attn, sT, mybir.ActivationFunctionType.Exp,
                            scale=scale)
                        gi = GRP_OFF[j] + grp
                        nc.vector.tensor_mul(
                            attn, attn, masks_sb[:, gi * 512:(gi + 1) * 512])
                        for s4 in range(4):
                            s = grp * 4 + s4
                            nc.tensor.matmul(
                                cd[:, 0:D + 1],
                                attn[:, s4 * 128:(s4 + 1) * 128],
                                v_sb[:, (s * HEADS + h) * (D + 1):
                                     (s * HEADS + h + 1) * (D + 1)],
                                start=(s == 0), stop=(s == nk - 1),
                                skip_group_check=True)
                    rcp = p_small.tile([128, 1], dt.float32, tag="rcp")
                    nc.vector.reciprocal(rcp, cd[:, D:D + 1])
                    ctxn = p_ctx.tile([128, 128], dt.bfloat16, tag="ctxn")
                    nc.vector.tensor_scalar_mul(ctxn, cd[:, 0:D], rcp)
                    ctxns.append(ctxn)
                    if h >= 1:
                        defer_outproj(h - 1)
                defer_outproj(HEADS - 1)
                out_t = p_out.tile([128, EMB], dt.float32, tag="out_t")
                nc.vector.tensor_copy(out_t[:, 0:1024], out_ps[:, 0:1024])
                nc.scalar.copy(out_t[:, 1024:2048], out_ps[:, 1024:2048])
                nc.sync.dma_start(
                    out=out[j * 128:(j + 1) * 128, :], in_=out_t)

    nc.finalize()
    return nc


def _shard_inputs(x, w_q, w_down, w_up_k, w_up_v, w_out, b_out):
    """Build the 8 per-core input maps (host-side layout prep)."""
    f32 = np.float32
    x = np.asarray(x, f32)
    wqT = np.ascontiguousarray(np.asarray(w_q, f32).T).astype(bf16)
    wq4 = np.ascontiguousarray(
        wqT.reshape(16, 128, 8, 256).transpose(2, 0, 1, 3))
    wdT = np.ascontiguousarray(np.asarray(w_down, f32).T).astype(bf16)
    wukT = np.ascontiguousarray(np.asarray(w_up_k, f32).T).astype(bf16)
    wuvT = np.ascontiguousarray(np.asarray(w_up_v, f32).T).astype(bf16)
    woT = np.ascontiguousarray(np.asarray(w_out, f32).T).astype(bf16)
    bias = np.asarray(b_out, f32).reshape(1, EMB).astype(bf16)

    xTs = [np.ascontiguousarray(x[b].T).astype(bf16) for b in range(B)]

    in_maps = []
    for c in range(NCORES):
        b, idx = c // 4, c % 4
        gs = [idx + 4 * j for j in range(QB)]
        xT = xTs[b]
        xTq = np.ascontiguousarray(
            np.concatenate([xT[:, g * 128:(g + 1) * 128] for g in gs], axis=1))
        # masks[10, 128, 512] per core: group gi covers slots s=grp*4+s4 of block j
        m = np.zeros((NGRP, 128, 512), dtype=bf16)
        tri = (np.arange(128)[:, None] <= np.arange(128)[None, :]).astype(bf16)
        onem = np.ones((128, 128), dtype=bf16)
        for j in range(QB):
            g = gs[j]
            for grp in range(NK[j] // 4):
                gi = GRP_OFF[j] + grp
                for s4 in range(4):
                    s = grp * 4 + s4
                    if s < g:
                        m[gi, :, s4 * 128:(s4 + 1) * 128] = onem
                    elif s == g:
                        m[gi, :, s4 * 128:(s4 + 1) * 128] = tri
        in_maps.append({
            "xT": xT, "xTq": xTq, "wdT": wdT, "wukT": wukT, "wuvT": wuvT,
            "wq4": wq4, "woT": woT, "bias": bias, "masks": m,
        })
    return in_maps


def _unshard(results, dtype):
    out = np.zeros((B, S, EMB), dtype=np.float32)
    for c in range(NCORES):
        b, idx = c // 4, c % 4
        o = results[c]["out"]
        for j in range(QB):
            g = idx + 4 * j
            out[b, g * 128:(g + 1) * 128, :] = o[j * 128:(j + 1) * 128, :]
    return out.astype(dtype)


def kernel(x, w_q, w_down, w_up_k, w_up_v, w_out, b_out):
    from concourse.bass_utils import run_bass_kernel_spmd
    if "nc" not in _CACHE:
        _CACHE["nc"] = build_program()
    nc = _CACHE["nc"]
    in_maps = _shard_inputs(x, w_q, w_down, w_up_k, w_up_v, w_out, b_out)
    res = run_bass_kernel_spmd(nc, in_maps, list(range(NCORES)))
    return _unshard(res.results, np.asarray(x).dtype)


if __name__ == "__main__":
    import reference
    inputs = {k: np.asarray(v) for k, v in reference.setup_inputs().items()}
    got = kernel(**inputs)
    want = np.asarray(reference.reference(**inputs))
    err = np.abs(got - want)
    print("absmax rel err:", err.max() / np.abs(want).max())


# revision 11
# speedup vs baseline: 1.4387x; 1.0282x over previous
"""Multi-Head Latent Attention kernel for 8 Trainium2 NeuronCores.

Sharding: data-parallel over (batch x strided query-block sets).
  core c: batch b = c // 4, idx = c % 4.
  Own query blocks (128 queries each): g = idx + 4*j, j in 0..3.
Each core redundantly computes latent/K/V for its batch (cross-core
collectives are ~30-60 GB/s here - far slower than recompute), so there is
zero cross-core communication. Causality is handled with a padded,
core-uniform block structure (NK(j) = 4j+4 key blocks for local block j)
plus per-core {0,1} multiplicative masks applied after exp - the SPMD
program is identical on all cores, only data differs.

All matmuls bf16 with fp32 PSUM accumulation; softmax runs without max
subtraction (scores are ~N(0,1) by construction, exp is safe in fp32).
"""

import math

import numpy as np
import ml_dtypes

import concourse.bacc as bacc
import concourse.mybir as mybir
import concourse.tile as tile

bf16 = ml_dtypes.bfloat16

EMB = 2048
HEADS = 16
D = 128          # head dim
L = 512          # latent dim
B, S = 2, 2048
NCORES = 8

EC = EMB // 128  # 16 e-chunks
LC = L // 128    # 4 l-chunks
QB = 4           # own q-blocks per core
NQ = QB * 128    # 512 own queries
SC = S // 512    # 4 s-chunks of 512
ST = S // 128    # 16 s-tiles of 128

NK = [4 * j + 4 for j in range(QB)]          # padded k-blocks per own block j
GRP_OFF = [0, 1, 3, 6]                        # mask group offset per j
NGRP = 10                                     # total [128,512] mask groups

_CACHE = {}


def build_program():
    nc = bacc.Bacc("TRN2", target_bir_lowering=False, debug=False)
    dt = mybir.dt

    xT = nc.dram_tensor("xT", [EMB, S], dt.bfloat16, kind="ExternalInput")
    xTq = nc.dram_tensor("xTq", [EMB, NQ], dt.bfloat16, kind="ExternalInput")
    wdT = nc.dram_tensor("wdT", [EMB, L], dt.bfloat16, kind="ExternalInput")
    wukT = nc.dram_tensor("wukT", [L, EMB], dt.bfloat16, kind="ExternalInput")
    wuvT = nc.dram_tensor("wuvT", [L, EMB], dt.bfloat16, kind="ExternalInput")
    # wq4[ftp, c, p, f]: f-tile pair ftp (256 f cols), e-chunk c, partition p
    wq4 = nc.dram_tensor("wq4", [EC // 2, EC, 128, 256], dt.bfloat16, kind="ExternalInput")
    woT = nc.dram_tensor("woT", [EMB, EMB], dt.bfloat16, kind="ExternalInput")
    bias = nc.dram_tensor("bias", [1, EMB], dt.bfloat16, kind="ExternalInput")
    masks = nc.dram_tensor("masks", [NGRP, 128, 512], dt.bfloat16, kind="ExternalInput")
    out = nc.dram_tensor("out", [NQ, EMB], dt.float32, kind="ExternalOutput")

    ident_t = nc.inline_tensor(np.eye(128, dtype=bf16), name="ident")
    ones_row_t = nc.inline_tensor(np.ones((1, 128), dtype=bf16), name="ones_row")

    scale = 1.0 / math.sqrt(D)
    import contextlib

    with tile.TileContext(nc) as tc, contextlib.ExitStack() as es:
        # ---- persistent (right-side) pools ----
        consts = es.enter_context(tc.tile_pool(name="consts", bufs=1, side="right"))
        p_qT = es.enter_context(tc.tile_pool(name="p_qT", bufs=1, side="right"))

        ident = consts.tile([128, 128], dt.bfloat16)
        nc.sync.dma_start(out=ident, in_=ident_t[:, :])
        ones_row = consts.tile([1, 128], dt.bfloat16)
        nc.sync.dma_start(out=ones_row, in_=ones_row_t[:, :])

        qT_sb = p_qT.tile([128, HEADS * NQ], dt.bfloat16)
        p_masks = es.enter_context(tc.tile_pool(name="p_masks", bufs=1, side="right"))
        masks_sb = p_masks.tile([128, NGRP * 512], dt.bfloat16)

        # left-stack pools, LIFO: latT > wuv > wuk > {xtq,wqs} > {wd,xt}
        lat_cm = tc.tile_pool(name="p_lat", bufs=1)
        p_lat = lat_cm.__enter__()
        latT_sb = p_lat.tile([128, LC * S], dt.bfloat16)

        wuv_cm = tc.tile_pool(name="p_wuv", bufs=1)
        p_wuv = wuv_cm.__enter__()
        wuv_sb = p_wuv.tile([128, LC * EMB], dt.bfloat16)

        wuk_cm = tc.tile_pool(name="p_wuk", bufs=1)
        p_wuk = wuk_cm.__enter__()
        wuk_sb = p_wuk.tile([128, LC * EMB], dt.bfloat16)

        wq_cm = tc.tile_pool(name="p_wq", bufs=2)
        p_wq = wq_cm.__enter__()
        xtq_sb = p_wq.tile([128, EC * NQ], dt.bfloat16, tag="xtq")

        ps0_cm = tc.tile_pool(name="ps0", bufs=4, space="PSUM")
        ps0 = ps0_cm.__enter__()

        # --- 0a: latentT[l, s] = wdT.T @ xT ---
        with tc.tile_pool(name="p_wd", bufs=1) as p_wd, \
             tc.tile_pool(name="p_xt", bufs=2) as p_xt:
            wd_sb = p_wd.tile([128, EC * L], dt.bfloat16)
            nc.sync.dma_start(
                out=wd_sb.rearrange("p (c l) -> p c l", c=EC),
                in_=wdT.rearrange("(c p) l -> p c l", p=128))
            for hc in range(2 * SC):
                xt = p_xt.tile([128, EC * 256], dt.bfloat16, tag="xt")
                nc.sync.dma_start(
                    out=xt.rearrange("p (c s) -> p c s", c=EC),
                    in_=xT[:, hc * 256:(hc + 1) * 256].rearrange(
                        "(c p) s -> p c s", p=128))
                # FIFO-paced prefetch on the sync ring: one ~1MB piece per panel
                if hc in (1, 2):
                    half = hc - 1
                    nc.sync.dma_start(
                        out=xtq_sb.rearrange("p (c q) -> p c q", c=EC)[
                            :, half * 8:(half + 1) * 8, :],
                        in_=xTq[half * 1024:(half + 1) * 1024, :].rearrange(
                            "(c p) q -> p c q", p=128))
                if hc in (3, 4):
                    half = hc - 3
                    nc.sync.dma_start(
                        out=wuk_sb.rearrange("p (c f) -> p c f", c=LC)[
                            :, half * 2:(half + 1) * 2, :],
                        in_=wukT[half * 256:(half + 1) * 256, :].rearrange(
                            "(c p) f -> p c f", p=128))
                if hc in (5, 6):
                    half = hc - 5
                    nc.sync.dma_start(
                        out=wuv_sb.rearrange("p (c f) -> p c f", c=LC)[
                            :, half * 2:(half + 1) * 2, :],
                        in_=wuvT[half * 256:(half + 1) * 256, :].rearrange(
                            "(c p) f -> p c f", p=128))
                if hc == 7:
                    nc.sync.dma_start(
                        out=masks_sb.rearrange("p (g q) -> p g q", g=NGRP),
                        in_=masks.rearrange("g p q -> p g q"))
                for lt in range(LC):
                    acc = ps0.tile([128, 256], dt.float32, tag="ps0")
                    for e in range(EC):
                        nc.tensor.matmul(
                            acc,
                            wd_sb[:, e * L + lt * 128: e * L + (lt + 1) * 128],
                            xt[:, e * 256:(e + 1) * 256],
                            start=(e == 0), stop=(e == EC - 1))
                    dst = latT_sb[:, lt * S + hc * 256: lt * S + (hc + 1) * 256]
                    if lt % 2 == 0:
                        nc.vector.tensor_copy(dst, acc)
                    else:
                        nc.scalar.copy(dst, acc)

        # --- 0d: qT[f, own q] = wq.T @ xTq  (wq streamed in f-tile pairs) ---
        for ftp in range(EC // 2):
            wqs = p_wq.tile([128, EC * 256], dt.bfloat16, tag="wqs")
            nc.sync.dma_start(
                out=wqs.rearrange("p (c f) -> p c f", c=EC),
                in_=wq4[ftp].rearrange("c p f -> p c f"))
            for fi in range(2):
                ft = 2 * ftp + fi
                acc = ps0.tile([128, NQ], dt.float32, tag="ps0")
                for e in range(EC):
                    nc.tensor.matmul(
                        acc,
                        wqs[:, e * 256 + fi * 128: e * 256 + (fi + 1) * 128],
                        xtq_sb[:, e * NQ:(e + 1) * NQ],
                        start=(e == 0), stop=(e == EC - 1))
                dst = qT_sb[:, ft * NQ:(ft + 1) * NQ]
                if ft % 2 == 0:
                    nc.vector.tensor_copy(dst, acc)
                else:
                    nc.scalar.copy(dst, acc)
        wq_cm.__exit__(None, None, None)
        ps0_cm.__exit__(None, None, None)
        psa_cm = tc.tile_pool(name="ps_acc", bufs=1, space="PSUM")
        psa = psa_cm.__enter__()

        # --- 0b: kT[f, s] = wukT.T @ latentT (weight-stationary over s-chunks) ---
        p_kT = es.enter_context(tc.tile_pool(name="p_kT", bufs=1, side="right"))
        kT_sb = p_kT.tile([128, HEADS * S], dt.bfloat16)
        for ft in range(EC):
            for lc in range(LC):
                accs = []
                for sc in range(SC):
                    if lc == 0:
                        acc = psa.tile([128, 512], dt.float32, tag=f"a{sc}")
                        accs.append(acc)
                    else:
                        acc = kacc[sc]
                    nc.tensor.matmul(
                        acc,
                        wuk_sb[:, lc * EMB + ft * 128: lc * EMB + (ft + 1) * 128],
                        latT_sb[:, lc * S + sc * 512: lc * S + (sc + 1) * 512],
                        start=(lc == 0), stop=(lc == LC - 1))
                if lc == 0:
                    kacc = accs
            for sc in range(SC):
                dst = kT_sb[:, ft * S + sc * 512: ft * S + (sc + 1) * 512]
                if (ft + sc) % 2 == 0:
                    nc.vector.tensor_copy(dst, kacc[sc])
                else:
                    nc.scalar.copy(dst, kacc[sc])
        wuk_cm.__exit__(None, None, None)

        # --- 0c: v[s, (h,129)] = latentT.T @ wuvT (+ ones col per head) ---
        p_v = es.enter_context(tc.tile_pool(name="p_v", bufs=1, side="right"))
        v_sb = p_v.tile([128, ST * HEADS * (D + 1)], dt.bfloat16)
        nc.vector.memset(
            v_sb.rearrange("p (t w) -> p t w", w=D + 1)[:, :, D:D + 1], 1.0)
        for st in range(ST):
            for lc in range(LC):
                accs = []
                for fc in range(SC):
                    if lc == 0:
                        acc = psa.tile([128, 512], dt.float32, tag=f"a{fc}")
                        accs.append(acc)
                    else:
                        acc = vacc[fc]
                    nc.tensor.matmul(
                        acc,
                        latT_sb[:, lc * S + st * 128: lc * S + (st + 1) * 128],
                        wuv_sb[:, lc * EMB + fc * 512: lc * EMB + (fc + 1) * 512],
                        start=(lc == 0), stop=(lc == LC - 1))
                if lc == 0:
                    vacc = accs
            for fc in range(SC):
                base = st * HEADS * (D + 1) + fc * 4 * (D + 1)
                dst = v_sb[:, base: base + 4 * (D + 1)].rearrange(
                    "p (h w) -> p h w", h=4)[:, :, 0:D]
                srcv = vacc[fc].rearrange("p (h w) -> p h w", h=4)
                if (st + fc) % 2 == 0:
                    nc.vector.tensor_copy(dst, srcv)
                else:
                    nc.scalar.copy(dst, srcv)
        wuv_cm.__exit__(None, None, None)
        lat_cm.__exit__(None, None, None)
        psa_cm.__exit__(None, None, None)

        # ============ phase 1: attention + out-proj ============
        with tc.tile_pool(name="p_bias", bufs=1) as p_bias, \
             tc.tile_pool(name="p_wo", bufs=4) as p_wo, \
             tc.tile_pool(name="p_attn", bufs=3) as p_attn, \
             tc.tile_pool(name="p_ctx", bufs=4) as p_ctx, \
             tc.tile_pool(name="p_small", bufs=4) as p_small, \
             tc.tile_pool(name="p_out", bufs=1) as p_out, \
             tc.tile_pool(name="ps_s", bufs=2, space="PSUM") as ps_s, \
             tc.tile_pool(name="ps_cd", bufs=2, space="PSUM") as ps_cd, \
             tc.tile_pool(name="ps_out", bufs=1, space="PSUM") as ps_out:

            bias_sb = p_bias.tile([1, EMB], dt.bfloat16)
            nc.sync.dma_start(out=bias_sb, in_=bias[:, :])

            for j in range(QB):
                nk = NK[j]
                out_ps = ps_out.tile([128, EMB], dt.float32, tag="out")
                for fc in range(4):
                    nc.tensor.matmul(
                        out_ps[:, fc * 512:(fc + 1) * 512],
                        ones_row,
                        bias_sb[:, fc * 512:(fc + 1) * 512],
                        start=True, stop=False, skip_group_check=True)
                ctxns = []

                def defer_outproj(h):
                    wo_h = wo_tiles[h]
                    ctxT_ps = ps_s.tile([128, 128], dt.bfloat16, tag="s")
                    nc.tensor.transpose(ctxT_ps, ctxns[h], ident)
                    ctxT = p_ctx.tile([128, 128], dt.bfloat16, tag="ctxT")
                    nc.scalar.copy(ctxT, ctxT_ps)
                    for fc in range(4):
                        nc.tensor.matmul(
                            out_ps[:, fc * 512:(fc + 1) * 512],
                            ctxT,
                            wo_h[:, fc * 512:(fc + 1) * 512],
                            start=False, stop=(h == HEADS - 1),
                            skip_group_check=True)

                wo_tiles = {}
                for h in range(HEADS):
                    wo_h = p_wo.tile([128, EMB], dt.bfloat16, tag="wo")
                    nc.sync.dma_start(out=wo_h, in_=woT[h * 128:(h + 1) * 128, :])
                    wo_tiles[h] = wo_h
                    cd = ps_cd.tile([128, 512], dt.float32, tag="cd")
                    for grp in range(nk // 4):
                        sT = ps_s.tile([128, 512], dt.float32, tag="s")
                        for s4 in range(4):
                            s = grp * 4 + s4
                            nc.tensor.matmul(
                                sT[:, s4 * 128:(s4 + 1) * 128],
                                kT_sb[:, h * S + s * 128: h * S + (s + 1) * 128],
                                qT_sb[:, h * NQ + j * 128: h * NQ + (j + 1) * 128],
                                start=(s4 == 0), stop=(s4 == 3),
                                skip_group_check=True)
                        attn = p_attn.tile([128, 512], dt.bfloat16, tag="attn")
                        nc.scalar.activation(
                            attn, sT, mybir.ActivationFunctionType.Exp,
                            scale=scale)
                        gi = GRP_OFF[j] + grp
                        nc.vector.tensor_mul(
                            attn, attn, masks_sb[:, gi * 512:(gi + 1) * 512])
                        for s4 in range(4):
                            s = grp * 4 + s4
                            nc.tensor.matmul(
                                cd[:, 0:D + 1],
                                attn[:, s4 * 128:(s4 + 1) * 128],
                                v_sb[:, (s * HEADS + h) * (D + 1):
                                     (s * HEADS + h + 1) * (D + 1)],
                                start=(s == 0), stop=(s == nk - 1),
                                skip_group_check=True)
                    rcp = p_small.tile([128, 1], dt.float32, tag="rcp")
                    nc.vector.reciprocal(rcp, cd[:, D:D + 1])
                    ctxn = p_ctx.tile([128, 128], dt.bfloat16, tag="ctxn")
                    nc.vector.tensor_scalar_mul(ctxn, cd[:, 0:D], rcp)
                    ctxns.append(ctxn)
                    if h >= 1:
                        defer_outproj(h - 1)
                defer_outproj(HEADS - 1)
                out_t = p_out.tile([128, EMB], dt.float32, tag="out_t")
                nc.vector.tensor_copy(out_t[:, 0:1024], out_ps[:, 0:1024])
                nc.scalar.copy(out_t[:, 1024:2048], out_ps[:, 1024:2048])
                nc.sync.dma_start(
                    out=out[j * 128:(j + 1) * 128, :], in_=out_t)

    nc.finalize()
    return nc


def _shard_inputs(x, w_q, w_down, w_up_k, w_up_v, w_out, b_out):
    """Build the 8 per-core input maps (host-side layout prep)."""
    f32 = np.float32
    x = np.asarray(x, f32)
    wqT = np.ascontiguousarray(np.asarray(w_q, f32).T).astype(bf16)
    wq4 = np.ascontiguousarray(
        wqT.reshape(16, 128, 8, 256).transpose(2, 0, 1, 3))
    wdT = np.ascontiguousarray(np.asarray(w_down, f32).T).astype(bf16)
    wukT = np.ascontiguousarray(np.asarray(w_up_k, f32).T).astype(bf16)
    wuvT = np.ascontiguousarray(np.asarray(w_up_v, f32).T).astype(bf16)
    woT = np.ascontiguousarray(np.asarray(w_out, f32).T).astype(bf16)
    bias = np.asarray(b_out, f32).reshape(1, EMB).astype(bf16)

    xTs = [np.ascontiguousarray(x[b].T).astype(bf16) for b in range(B)]

    in_maps = []
    for c in range(NCORES):
        b, idx = c // 4, c % 4
        gs = [idx + 4 * j for j in range(QB)]
        xT = xTs[b]
        xTq = np.ascontiguousarray(
            np.concatenate([xT[:, g * 128:(g + 1) * 128] for g in gs], axis=1))
        # masks[10, 128, 512] per core: group gi covers slots s=grp*4+s4 of block j
        m = np.zeros((NGRP, 128, 512), dtype=bf16)
        tri = (np.arange(128)[:, None] <= np.arange(128)[None, :]).astype(bf16)
        onem = np.ones((128, 128), dtype=bf16)
        for j in range(QB):
            g = gs[j]
            for grp in range(NK[j] // 4):
                gi = GRP_OFF[j] + grp
                for s4 in range(4):
                    s = grp * 4 + s4
                    if s < g:
                        m[gi, :, s4 * 128:(s4 + 1) * 128] = onem
                    elif s == g:
                        m[gi, :, s4 * 128:(s4 + 1) * 128] = tri
        in_maps.append({
            "xT": xT, "xTq": xTq, "wdT": wdT, "wukT": wukT, "wuvT": wuvT,
            "wq4": wq4, "woT": woT, "bias": bias, "masks": m,
        })
    return in_maps


def _unshard(results, dtype):
    out = np.zeros((B, S, EMB), dtype=np.float32)
    for c in range(NCORES):
        b, idx = c // 4, c % 4
        o = results[c]["out"]
        for j in range(QB):
            g = idx + 4 * j
            out[b, g * 128:(g + 1) * 128, :] = o[j * 128:(j + 1) * 128, :]
    return out.astype(dtype)


def kernel(x, w_q, w_down, w_up_k, w_up_v, w_out, b_out):
    from concourse.bass_utils import run_bass_kernel_spmd
    if "nc" not in _CACHE:
        _CACHE["nc"] = build_program()
    nc = _CACHE["nc"]
    in_maps = _shard_inputs(x, w_q, w_down, w_up_k, w_up_v, w_out, b_out)
    res = run_bass_kernel_spmd(nc, in_maps, list(range(NCORES)))
    return _unshard(res.results, np.asarray(x).dtype)


if __name__ == "__main__":
    import reference
    inputs = {k: np.asarray(v) for k, v in reference.setup_inputs().items()}
    got = kernel(**inputs)
    want = np.asarray(reference.reference(**inputs))
    err = np.abs(got - want)
    print("absmax rel err:", err.max() / np.abs(want).max())


# revision 12
# speedup vs baseline: 1.4441x; 1.0038x over previous
"""Multi-Head Latent Attention kernel for 8 Trainium2 NeuronCores.

Sharding: data-parallel over (batch x strided query-block sets).
  core c: batch b = c // 4, idx = c % 4.
  Own query blocks (128 queries each): g = idx + 4*j, j in 0..3.
Each core redundantly computes latent/K/V for its batch (cross-core
collectives are ~30-60 GB/s here - far slower than recompute), so there is
zero cross-core communication. Causality is handled with a padded,
core-uniform block structure (NK(j) = 4j+4 key blocks for local block j)
plus per-core {0,1} multiplicative masks applied after exp - the SPMD
program is identical on all cores, only data differs.

All matmuls bf16 with fp32 PSUM accumulation; softmax runs without max
subtraction (scores are ~N(0,1) by construction, exp is safe in fp32).
"""

import math

import numpy as np
import ml_dtypes

import concourse.bacc as bacc
import concourse.mybir as mybir
import concourse.tile as tile

bf16 = ml_dtypes.bfloat16

EMB = 2048
HEADS = 16
D = 128          # head dim
L = 512          # latent dim
B, S = 2, 2048
NCORES = 8

EC = EMB // 128  # 16 e-chunks
LC = L // 128    # 4 l-chunks
QB = 4           # own q-blocks per core
NQ = QB * 128    # 512 own queries
SC = S // 512    # 4 s-chunks of 512
ST = S // 128    # 16 s-tiles of 128

NK = [4 * j + 4 for j in range(QB)]          # padded k-blocks per own block j
GRP_OFF = [0, 1, 3, 6]                        # mask group offset per j
NGRP = 10                                     # total [128,512] mask groups

_CACHE = {}


def build_program():
    nc = bacc.Bacc("TRN2", target_bir_lowering=False, debug=False)
    dt = mybir.dt

    xT = nc.dram_tensor("xT", [EMB, S], dt.bfloat16, kind="ExternalInput")
    xTq = nc.dram_tensor("xTq", [EMB, NQ], dt.bfloat16, kind="ExternalInput")
    wdT = nc.dram_tensor("wdT", [EMB, L], dt.bfloat16, kind="ExternalInput")
    wukT = nc.dram_tensor("wukT", [L, EMB], dt.bfloat16, kind="ExternalInput")
    wuvT = nc.dram_tensor("wuvT", [L, EMB], dt.bfloat16, kind="ExternalInput")
    # wq4[ftp, c, p, f]: f-tile pair ftp (256 f cols), e-chunk c, partition p
    wq4 = nc.dram_tensor("wq4", [EC // 2, EC, 128, 256], dt.bfloat16, kind="ExternalInput")
    woT = nc.dram_tensor("woT", [EMB, EMB], dt.bfloat16, kind="ExternalInput")
    bias = nc.dram_tensor("bias", [1, EMB], dt.bfloat16, kind="ExternalInput")
    masks = nc.dram_tensor("masks", [NGRP, 128, 512], dt.bfloat16, kind="ExternalInput")
    out = nc.dram_tensor("out", [NQ, EMB], dt.float32, kind="ExternalOutput")

    ident_t = nc.inline_tensor(np.eye(128, dtype=bf16), name="ident")
    ones_row_t = nc.inline_tensor(np.ones((1, 128), dtype=bf16), name="ones_row")

    scale = 1.0 / math.sqrt(D)
    import contextlib

    with tile.TileContext(nc) as tc, contextlib.ExitStack() as es:
        # ---- persistent (right-side) pools ----
        consts = es.enter_context(tc.tile_pool(name="consts", bufs=1, side="right"))
        p_qT = es.enter_context(tc.tile_pool(name="p_qT", bufs=1, side="right"))

        ident = consts.tile([128, 128], dt.bfloat16)
        nc.sync.dma_start(out=ident, in_=ident_t[:, :])
        ones_row = consts.tile([1, 128], dt.bfloat16)
        nc.sync.dma_start(out=ones_row, in_=ones_row_t[:, :])

        qT_sb = p_qT.tile([128, HEADS * NQ], dt.bfloat16)
        p_masks = es.enter_context(tc.tile_pool(name="p_masks", bufs=1, side="right"))
        masks_sb = p_masks.tile([128, NGRP * 512], dt.bfloat16)

        # left-stack pools, LIFO: latT > wuv > wuk > {xtq,wqs} > {wd,xt}
        lat_cm = tc.tile_pool(name="p_lat", bufs=1)
        p_lat = lat_cm.__enter__()
        latT_sb = p_lat.tile([128, LC * S], dt.bfloat16)

        wuv_cm = tc.tile_pool(name="p_wuv", bufs=1)
        p_wuv = wuv_cm.__enter__()
        wuv_sb = p_wuv.tile([128, LC * EMB], dt.bfloat16)

        wuk_cm = tc.tile_pool(name="p_wuk", bufs=1)
        p_wuk = wuk_cm.__enter__()
        wuk_sb = p_wuk.tile([128, LC * EMB], dt.bfloat16)

        wq_cm = tc.tile_pool(name="p_wq", bufs=2)
        p_wq = wq_cm.__enter__()
        xtq_sb = p_wq.tile([128, EC * NQ], dt.bfloat16, tag="xtq")

        ps0_cm = tc.tile_pool(name="ps0", bufs=4, space="PSUM")
        ps0 = ps0_cm.__enter__()

        # --- 0a: latentT[l, s] = wdT.T @ xT ---
        with tc.tile_pool(name="p_wd", bufs=1) as p_wd, \
             tc.tile_pool(name="p_xt", bufs=2) as p_xt:
            wd_sb = p_wd.tile([128, EC * L], dt.bfloat16)
            for q4 in range(4):
                nc.sync.dma_start(
                    out=wd_sb.rearrange("p (c l) -> p c l", c=EC)[
                        :, q4 * 4:(q4 + 1) * 4, :],
                    in_=wdT[q4 * 512:(q4 + 1) * 512, :].rearrange(
                        "(c p) l -> p c l", p=128))
            for hc in range(2 * SC):
                xt = p_xt.tile([128, EC * 256], dt.bfloat16, tag="xt")
                nparts = 2 if hc == 0 else 1
                for pp in range(nparts):
                    w = EC // nparts
                    nc.sync.dma_start(
                        out=xt.rearrange("p (c s) -> p c s", c=EC)[
                            :, pp * w:(pp + 1) * w, :],
                        in_=xT[pp * w * 128:(pp + 1) * w * 128,
                               hc * 256:(hc + 1) * 256].rearrange(
                            "(c p) s -> p c s", p=128))
                # FIFO-paced prefetch on the sync ring: one ~1MB piece per panel
                if hc in (1, 2):
                    half = hc - 1
                    nc.sync.dma_start(
                        out=xtq_sb.rearrange("p (c q) -> p c q", c=EC)[
                            :, half * 8:(half + 1) * 8, :],
                        in_=xTq[half * 1024:(half + 1) * 1024, :].rearrange(
                            "(c p) q -> p c q", p=128))
                if hc in (3, 4):
                    half = hc - 3
                    nc.sync.dma_start(
                        out=wuk_sb.rearrange("p (c f) -> p c f", c=LC)[
                            :, half * 2:(half + 1) * 2, :],
                        in_=wukT[half * 256:(half + 1) * 256, :].rearrange(
                            "(c p) f -> p c f", p=128))
                if hc in (5, 6):
                    half = hc - 5
                    nc.sync.dma_start(
                        out=wuv_sb.rearrange("p (c f) -> p c f", c=LC)[
                            :, half * 2:(half + 1) * 2, :],
                        in_=wuvT[half * 256:(half + 1) * 256, :].rearrange(
                            "(c p) f -> p c f", p=128))
                if hc == 7:
                    nc.sync.dma_start(
                        out=masks_sb.rearrange("p (g q) -> p g q", g=NGRP),
                        in_=masks.rearrange("g p q -> p g q"))
                for lt in range(LC):
                    acc = ps0.tile([128, 256], dt.float32, tag="ps0")
                    for e in range(EC):
                        nc.tensor.matmul(
                            acc,
                            wd_sb[:, e * L + lt * 128: e * L + (lt + 1) * 128],
                            xt[:, e * 256:(e + 1) * 256],
                            start=(e == 0), stop=(e == EC - 1))
                    dst = latT_sb[:, lt * S + hc * 256: lt * S + (hc + 1) * 256]
                    if lt % 2 == 0:
                        nc.vector.tensor_copy(dst, acc)
                    else:
                        nc.scalar.copy(dst, acc)

        # --- 0d: qT[f, own q] = wq.T @ xTq  (wq streamed in f-tile pairs) ---
        for ftp in range(EC // 2):
            wqs = p_wq.tile([128, EC * 256], dt.bfloat16, tag="wqs")
            nc.sync.dma_start(
                out=wqs.rearrange("p (c f) -> p c f", c=EC),
                in_=wq4[ftp].rearrange("c p f -> p c f"))
            for fi in range(2):
                ft = 2 * ftp + fi
                acc = ps0.tile([128, NQ], dt.float32, tag="ps0")
                for e in range(EC):
                    nc.tensor.matmul(
                        acc,
                        wqs[:, e * 256 + fi * 128: e * 256 + (fi + 1) * 128],
                        xtq_sb[:, e * NQ:(e + 1) * NQ],
                        start=(e == 0), stop=(e == EC - 1))
                dst = qT_sb[:, ft * NQ:(ft + 1) * NQ]
                if ft % 2 == 0:
                    nc.vector.tensor_copy(dst, acc)
                else:
                    nc.scalar.copy(dst, acc)
        wq_cm.__exit__(None, None, None)
        ps0_cm.__exit__(None, None, None)
        psa_cm = tc.tile_pool(name="ps_acc", bufs=1, space="PSUM")
        psa = psa_cm.__enter__()

        # --- 0b: kT[f, s] = wukT.T @ latentT (weight-stationary over s-chunks) ---
        p_kT = es.enter_context(tc.tile_pool(name="p_kT", bufs=1, side="right"))
        kT_sb = p_kT.tile([128, HEADS * S], dt.bfloat16)
        for ft in range(EC):
            for lc in range(LC):
                accs = []
                for sc in range(SC):
                    if lc == 0:
                        acc = psa.tile([128, 512], dt.float32, tag=f"a{sc}")
                        accs.append(acc)
                    else:
                        acc = kacc[sc]
                    nc.tensor.matmul(
                        acc,
                        wuk_sb[:, lc * EMB + ft * 128: lc * EMB + (ft + 1) * 128],
                        latT_sb[:, lc * S + sc * 512: lc * S + (sc + 1) * 512],
                        start=(lc == 0), stop=(lc == LC - 1))
                if lc == 0:
                    kacc = accs
            for sc in range(SC):
                dst = kT_sb[:, ft * S + sc * 512: ft * S + (sc + 1) * 512]
                if (ft + sc) % 2 == 0:
                    nc.vector.tensor_copy(dst, kacc[sc])
                else:
                    nc.scalar.copy(dst, kacc[sc])
        wuk_cm.__exit__(None, None, None)

        # --- 0c: v[s, (h,129)] = latentT.T @ wuvT (+ ones col per head) ---
        p_v = es.enter_context(tc.tile_pool(name="p_v", bufs=1, side="right"))
        v_sb = p_v.tile([128, ST * HEADS * (D + 1)], dt.bfloat16)
        nc.vector.memset(
            v_sb.rearrange("p (t w) -> p t w", w=D + 1)[:, :, D:D + 1], 1.0)
        for st in range(ST):
            for lc in range(LC):
                accs = []
                for fc in range(SC):
                    if lc == 0:
                        acc = psa.tile([128, 512], dt.float32, tag=f"a{fc}")
                        accs.append(acc)
                    else:
                        acc = vacc[fc]
                    nc.tensor.matmul(
                        acc,
                        latT_sb[:, lc * S + st * 128: lc * S + (st + 1) * 128],
                        wuv_sb[:, lc * EMB + fc * 512: lc * EMB + (fc + 1) * 512],
                        start=(lc == 0), stop=(lc == LC - 1))
                if lc == 0:
                    vacc = accs
            for fc in range(SC):
                base = st * HEADS * (D + 1) + fc * 4 * (D + 1)
                dst = v_sb[:, base: base + 4 * (D + 1)].rearrange(
                    "p (h w) -> p h w", h=4)[:, :, 0:D]
                srcv = vacc[fc].rearrange("p (h w) -> p h w", h=4)
                if (st + fc) % 2 == 0:
                    nc.vector.tensor_copy(dst, srcv)
                else:
                    nc.scalar.copy(dst, srcv)
        wuv_cm.__exit__(None, None, None)
        lat_cm.__exit__(None, None, None)
        psa_cm.__exit__(None, None, None)

        # ============ phase 1: attention + out-proj ============
        with tc.tile_pool(name="p_bias", bufs=1) as p_bias, \
             tc.tile_pool(name="p_wo", bufs=4) as p_wo, \
             tc.tile_pool(name="p_attn", bufs=3) as p_attn, \
             tc.tile_pool(name="p_ctx", bufs=4) as p_ctx, \
             tc.tile_pool(name="p_small", bufs=4) as p_small, \
             tc.tile_pool(name="p_out", bufs=1) as p_out, \
             tc.tile_pool(name="ps_s", bufs=2, space="PSUM") as ps_s, \
             tc.tile_pool(name="ps_cd", bufs=2, space="PSUM") as ps_cd, \
             tc.tile_pool(name="ps_out", bufs=1, space="PSUM") as ps_out:

            bias_sb = p_bias.tile([1, EMB], dt.bfloat16)
            nc.sync.dma_start(out=bias_sb, in_=bias[:, :])

            for j in range(QB):
                nk = NK[j]
                out_ps = ps_out.tile([128, EMB], dt.float32, tag="out")
                for fc in range(4):
                    nc.tensor.matmul(
                        out_ps[:, fc * 512:(fc + 1) * 512],
                        ones_row,
                        bias_sb[:, fc * 512:(fc + 1) * 512],
                        start=True, stop=False, skip_group_check=True)
                ctxns = []

                def defer_outproj(h):
                    wo_h = wo_tiles[h]
                    ctxT_ps = ps_s.tile([128, 128], dt.bfloat16, tag="s")
                    nc.tensor.transpose(ctxT_ps, ctxns[h], ident)
                    ctxT = p_ctx.tile([128, 128], dt.bfloat16, tag="ctxT")
                    nc.scalar.copy(ctxT, ctxT_ps)
                    for fc in range(4):
                        nc.tensor.matmul(
                            out_ps[:, fc * 512:(fc + 1) * 512],
                            ctxT,
                            wo_h[:, fc * 512:(fc + 1) * 512],
                            start=False, stop=(h == HEADS - 1),
                            skip_group_check=True)

                wo_tiles = {}
                for h in range(HEADS):
                    wo_h = p_wo.tile([128, EMB], dt.bfloat16, tag="wo")
                    nc.sync.dma_start(out=wo_h, in_=woT[h * 128:(h + 1) * 128, :])
                    wo_tiles[h] = wo_h
                    cd = ps_cd.tile([128, 512], dt.float32, tag="cd")
                    for grp in range(nk // 4):
                        sT = ps_s.tile([128, 512], dt.float32, tag="s")
                        for s4 in range(4):
                            s = grp * 4 + s4
                            nc.tensor.matmul(
                                sT[:, s4 * 128:(s4 + 1) * 128],
                                kT_sb[:, h * S + s * 128: h * S + (s + 1) * 128],
                                qT_sb[:, h * NQ + j * 128: h * NQ + (j + 1) * 128],
                                start=(s4 == 0), stop=(s4 == 3),
                                skip_group_check=True)
                        attn = p_attn.tile([128, 512], dt.bfloat16, tag="attn")
                        nc.scalar.activation(
                            attn, sT, mybir.ActivationFunctionType.Exp,
                            scale=scale)
                        gi = GRP_OFF[j] + grp
                        nc.vector.tensor_mul(
                            attn, attn, masks_sb[:, gi * 512:(gi + 1) * 512])
                        for s4 in range(4):
                            s = grp * 4 + s4
                            nc.tensor.matmul(
                                cd[:, 0:D + 1],
                                attn[:, s4 * 128:(s4 + 1) * 128],
                                v_sb[:, (s * HEADS + h) * (D + 1):
                                     (s * HEADS + h + 1) * (D + 1)],
                                start=(s == 0), stop=(s == nk - 1),
                                skip_group_check=True)
                    rcp = p_small.tile([128, 1], dt.float32, tag="rcp")
                    nc.vector.reciprocal(rcp, cd[:, D:D + 1])
                    ctxn = p_ctx.tile([128, 128], dt.bfloat16, tag="ctxn")
                    nc.vector.tensor_scalar_mul(ctxn, cd[:, 0:D], rcp)
                    ctxns.append(ctxn)
                    if h >= 1:
                        defer_outproj(h - 1)
                defer_outproj(HEADS - 1)
                out_t = p_out.tile([128, EMB], dt.float32, tag="out_t")
                for fc in range(4):
                    sl = slice(fc * 512, (fc + 1) * 512)
                    if fc % 2 == 0:
                        nc.vector.tensor_copy(out_t[:, sl], out_ps[:, sl])
                    else:
                        nc.scalar.copy(out_t[:, sl], out_ps[:, sl])
                nc.sync.dma_start(
                    out=out[j * 128:(j + 1) * 128, :], in_=out_t)

    nc.finalize()
    return nc


def _shard_inputs(x, w_q, w_down, w_up_k, w_up_v, w_out, b_out):
    """Build the 8 per-core input maps (host-side layout prep)."""
    f32 = np.float32
    x = np.asarray(x, f32)
    wqT = np.ascontiguousarray(np.asarray(w_q, f32).T).astype(bf16)
    wq4 = np.ascontiguousarray(
        wqT.reshape(16, 128, 8, 256).transpose(2, 0, 1, 3))
    wdT = np.ascontiguousarray(np.asarray(w_down, f32).T).astype(bf16)
    wukT = np.ascontiguousarray(np.asarray(w_up_k, f32).T).astype(bf16)
    wuvT = np.ascontiguousarray(np.asarray(w_up_v, f32).T).astype(bf16)
    woT = np.ascontiguousarray(np.asarray(w_out, f32).T).astype(bf16)
    bias = np.asarray(b_out, f32).reshape(1, EMB).astype(bf16)

    xTs = [np.ascontiguousarray(x[b].T).astype(bf16) for b in range(B)]

    in_maps = []
    for c in range(NCORES):
        b, idx = c // 4, c % 4
        gs = [idx + 4 * j for j in range(QB)]
        xT = xTs[b]
        xTq = np.ascontiguousarray(
            np.concatenate([xT[:, g * 128:(g + 1) * 128] for g in gs], axis=1))
        # masks[10, 128, 512] per core: group gi covers slots s=grp*4+s4 of block j
        m = np.zeros((NGRP, 128, 512), dtype=bf16)
        tri = (np.arange(128)[:, None] <= np.arange(128)[None, :]).astype(bf16)
        onem = np.ones((128, 128), dtype=bf16)
        for j in range(QB):
            g = gs[j]
            for grp in range(NK[j] // 4):
                gi = GRP_OFF[j] + grp
                for s4 in range(4):
                    s = grp * 4 + s4
                    if s < g:
                        m[gi, :, s4 * 128:(s4 + 1) * 128] = onem
                    elif s == g:
                        m[gi, :, s4 * 128:(s4 + 1) * 128] = tri
        in_maps.append({
            "xT": xT, "xTq": xTq, "wdT": wdT, "wukT": wukT, "wuvT": wuvT,
            "wq4": wq4, "woT": woT, "bias": bias, "masks": m,
        })
    return in_maps


def _unshard(results, dtype):
    out = np.zeros((B, S, EMB), dtype=np.float32)
    for c in range(NCORES):
        b, idx = c // 4, c % 4
        o = results[c]["out"]
        for j in range(QB):
            g = idx + 4 * j
            out[b, g * 128:(g + 1) * 128, :] = o[j * 128:(j + 1) * 128, :]
    return out.astype(dtype)


def kernel(x, w_q, w_down, w_up_k, w_up_v, w_out, b_out):
    from concourse.bass_utils import run_bass_kernel_spmd
    if "nc" not in _CACHE:
        _CACHE["nc"] = build_program()
    nc = _CACHE["nc"]
    in_maps = _shard_inputs(x, w_q, w_down, w_up_k, w_up_v, w_out, b_out)
    res = run_bass_kernel_spmd(nc, in_maps, list(range(NCORES)))
    return _unshard(res.results, np.asarray(x).dtype)


if __name__ == "__main__":
    import reference
    inputs = {k: np.asarray(v) for k, v in reference.setup_inputs().items()}
    got = kernel(**inputs)
    want = np.asarray(reference.reference(**inputs))
    err = np.abs(got - want)
    print("absmax rel err:", err.max() / np.abs(want).max())
